# revision 51
# baseline (speedup 1.0000x reference)
"""Dark-Channel-Prior dehazing (DCPGenerator) Trainium2 Bass kernel, v9.

v8 -> v9: the guided filter runs as a fast-guided-filter at 2x subsample
(256x256): all six box filters (I, II, p, Ip, a, b), the cov/var/a/b
math, and the vbox matmuls operate on 1/4 the pixels with radius-20
bands; mean_a/mean_b are bilinearly upsampled (PE matmuls for rows, DVE
for columns) and T = mean_a*I + mean_b is applied at full resolution.
Subsampling of guid / pooled-dark runs on the PE with selection
matrices.  Dark channel, top-k secant, and A estimation stay full-res.
"""
import numpy as np
from contextlib import ExitStack

H = 512
W = 512
NCHUNK = 4
CW = 512
NW = NCHUNK * CW            # 2048
PADW = 526                  # 7 | 512 | 7
WIN_PAD = 7
RADIUS = 40
# sub-grid (fast guided filter, s=2)
HS = 256
WS = 256
NCS = 2
RS = 20
LEAD = 24                   # leading zeros in sub scan layout (>=RS+1)
SEG = 300                   # WS + 44-zero gap (>= 2*RS+1)
SCN_W = LEAD + NCS * SEG    # 624
NWS = NCS * WS              # 512
EPS = 1e-3
OMEGA = 0.95
TOPN = int(0.01 * H * W)    # 2621
T0 = 0.0055
T1 = 0.0085
BAND = 2e-4
SECANT_ROUNDS = 5

_CACHE = {}


def _host_consts():
    # full-res H-direction box weights are no longer needed; sub-grid ones:
    i = np.arange(HS)
    n1s = np.minimum(i + RS, HS - 1) - np.maximum(i - RS, 0) + 1
    inv_ns = (1.0 / n1s).astype(np.float32)
    k = np.arange(128)[:, None]
    p = np.arange(128)[None, :]
    bands = (np.abs(k - p) <= RS).astype(np.float32)
    bus = (k >= p + 128 - RS).astype(np.float32) / 41.0 / 41.0
    bds = (k <= p - (128 - RS)).astype(np.float32) / 41.0 / 41.0
    bm0s = bands * inv_ns[0:128][None, :] / 41.0
    bm1s = bands * inv_ns[128:256][None, :] / 41.0
    fix40 = np.concatenate([41.0 * inv_ns[0:RS], 41.0 * inv_ns[WS - RS:]])
    fixs = np.tile(fix40[None, :], (128, NCS)).copy()        # [128, 80]
    ident = np.eye(128, dtype=np.float32)
    # row-subsample selection: out q <- full partition 2q (two half matrices)
    selA = np.zeros((128, 128), np.float32)
    selB = np.zeros((128, 128), np.float32)
    for q in range(64):
        selA[2 * q, q] = 1.0
    for q in range(64, 128):
        selB[2 * (q - 64), q] = 1.0
    # row-upsample (bilinear, sub sample i at full row 2i)
    U = {}
    for c in range(NCHUNK):
        for q in range(128):
            r = 128 * c + q
            if r % 2 == 0:
                pairs = [(r // 2, 1.0)]
            else:
                i0 = (r - 1) // 2
                i1 = min(i0 + 1, HS - 1)
                pairs = [(i0, 0.5), (i1, 0.5)] if i1 != i0 else [(i0, 1.0)]
            for i_, wgt in pairs:
                sc, pp_ = divmod(i_, 128)
                U.setdefault((c, sc), np.zeros((128, 128), np.float32))[
                    pp_, q] += wgt
    return {"bm0s": bm0s, "bm1s": bm1s, "bus": bus, "bds": bds,
            "fixs": fixs, "ident": ident, "selA": selA, "selB": selB,
            "u00": U[(0, 0)], "u10": U[(1, 0)], "u11": U[(1, 1)],
            "u21": U[(2, 1)], "u31": U[(3, 1)]}


def _build():
    import concourse.bacc as bacc
    import concourse.tile as tile
    import concourse.bass as bass
    from concourse import mybir

    f32 = mybir.dt.float32
    f32r = mybir.dt.float32r
    bf16 = mybir.dt.bfloat16
    Alu = mybir.AluOpType
    Act = mybir.ActivationFunctionType

    nc = bacc.Bacc("TRN2", target_bir_lowering=False, debug=False, num_devices=8)
    V = nc.vector
    G = nc.gpsimd

    x_ext = nc.dram_tensor("x", [2, 3, H, W], f32, kind="ExternalInput").ap()
    c128_names = ("bm0s", "bm1s", "bus", "bds", "ident", "selA", "selB",
                  "u00", "u10", "u11", "u21", "u31")
    c128_exts = {nm: nc.dram_tensor(nm, [128, 128], f32, kind="ExternalInput").ap()
                 for nm in c128_names}
    fixs_ext = nc.dram_tensor("fixs", [128, NCS * 2 * RS], f32,
                              kind="ExternalInput").ap()
    y_ext = nc.dram_tensor("y", [2, 3, H, W], f32, kind="ExternalOutput").ap()

    def cview(t, width=CW):
        return t.rearrange("p (c w) -> p c w", w=width)

    def fbcast(ap_col, n):
        return bass.AP(tensor=ap_col.tensor, offset=ap_col.offset,
                       ap=[list(p) for p in ap_col.ap[:-1]] + [[0, n]])

    def segview(t, off, c0=0, nch=NCS):
        """[128, nch, WS] view into a [128, SCN_W] sub tile."""
        base = t[:]
        return bass.AP(tensor=base.tensor, offset=base.offset + off + c0 * SEG,
                       ap=[list(base.ap[0]), [SEG, nch], [1, WS]])

    def sview(t):
        return segview(t, LEAD)

    with ExitStack() as ctx:
        tc = ctx.enter_context(tile.TileContext(nc))

        cpool = ctx.enter_context(tc.tile_pool(name="cpool", bufs=1))
        srcp = ctx.enter_context(tc.tile_pool(name="srcp", bufs=1))
        scn = ctx.enter_context(tc.tile_pool(name="scn", bufs=1))
        pp = ctx.enter_context(tc.tile_pool(name="pp", bufs=1))
        cump = ctx.enter_context(tc.tile_pool(name="cump", bufs=2))
        boxes = ctx.enter_context(tc.tile_pool(name="boxes", bufs=1))
        rot = ctx.enter_context(tc.tile_pool(name="rot", bufs=2))
        mrot = ctx.enter_context(tc.tile_pool(name="mrot", bufs=4))
        abt = ctx.enter_context(tc.tile_pool(name="abt", bufs=3))
        sab = ctx.enter_context(tc.tile_pool(name="sab", bufs=3))
        dout = ctx.enter_context(tc.tile_pool(name="dout", bufs=2))
        mfull = ctx.enter_context(tc.tile_pool(name="mfull", bufs=2))
        tiny = ctx.enter_context(tc.tile_pool(name="tiny", bufs=1))
        pbig = ctx.enter_context(tc.tile_pool(name="pbig", bufs=1, space="PSUM"))
        pmid = ctx.enter_context(tc.tile_pool(name="pmid", bufs=2, space="PSUM"))
        psml = ctx.enter_context(tc.tile_pool(name="psml", bufs=1, space="PSUM"))

        # ------------------------------------- constants (loaded after x DMAs)
        cbf = {}
        stage = cpool.tile([128, 128], f32, name="s_band")
        for nm in ("bm0s", "bm1s", "bus", "bds"):
            cbf[nm] = cpool.tile([128, 128], f32r, name=f"c_{nm}")
        for nm in ("ident", "selA", "selB", "u00", "u10", "u11", "u21", "u31"):
            cbf[nm] = cpool.tile([128, 128], bf16, name=f"c_{nm}")
        c_fixs = cpool.tile([128, NCS * 2 * RS], f32, name="c_fixs")
        c_ones128 = cpool.tile([128, 1], f32, name="c_ones128")
        c_ones1x = cpool.tile([1, 128], f32, name="c_ones1x")

        def load_consts():
            for nm in ("bm0s", "bm1s", "bus", "bds", "ident", "selA", "selB",
                       "u00", "u10", "u11", "u21", "u31"):
                nc.sync.dma_start(out=stage[:], in_=c128_exts[nm][:])
                nc.scalar.copy(cbf[nm][:], stage[:])
            nc.sync.dma_start(out=c_fixs[:], in_=fixs_ext[:])
            V.memset(c_ones128[:], 1.0)
            V.memset(c_ones1x[:], 1.0)

        # --------------------------------------------------- persistent tiles
        x16 = [[srcp.tile([128, NW], bf16, name=f"x16_{s}_{c}")
                for c in range(3)] for s in range(2)]
        t_guid = [srcp.tile([128, NW], bf16, name=f"guid{s}") for s in range(2)]
        # sub-grid scan-layout sources (f32): I, p, Ip, II, a, b per sample
        t_is = [scn.tile([128, SCN_W], f32, name=f"is{s}") for s in range(2)]
        t_ps = [scn.tile([128, SCN_W], f32, name=f"ps{s}") for s in range(2)]
        t_ip = [scn.tile([128, SCN_W], f32, name=f"ip{s}") for s in range(2)]
        t_ii = [scn.tile([128, SCN_W], f32, name=f"ii{s}") for s in range(2)]
        mxp = pp.tile([128, NCHUNK * PADW], bf16, name="mxp")
        w1 = pp.tile([128, NCHUNK * PADW], bf16, name="w1")
        uhTp = pp.tile([128, NCHUNK * PADW], bf16, name="uhTp")
        poolT = pp.tile([128, NW], bf16, name="poolT")
        uh = [pp.tile([128, NW], bf16, name=f"uh{s}") for s in range(2)]
        mean_Is = [boxes.tile([128, NWS], f32, name=f"meanIs{s}")
                   for s in range(2)]
        rec_s = [boxes.tile([128, NWS], f32, name=f"recs{s}") for s in range(2)]

        junk = w1[:, 0:NW]
        junk_c = junk.rearrange("p (c w) -> p c w", w=CW)

        # zero the sub scan-layout gaps once
        for t in (t_is[0], t_is[1], t_ps[0], t_ps[1], t_ip[0], t_ip[1],
                  t_ii[0], t_ii[1]):
            V.memset(t[:, 0:LEAD], 0.0)
            for c in range(NCS):
                V.memset(t[:, LEAD + c * SEG + WS: LEAD + (c + 1) * SEG], 0.0)

        # ---------------------------------------------------------- helpers
        def interior(t):
            return cview(t, PADW)[:, :, WIN_PAD:WIN_PAD + CW]

        def memset_pads(t, eng):
            v = cview(t, PADW)
            for c in range(NCHUNK):
                eng.memset(v[:, c, 0:WIN_PAD], 1.0)
                eng.memset(v[:, c, PADW - WIN_PAD:PADW], 1.0)

        def hpool(dst, padded, scratch):
            a = cview(padded, PADW)
            b = cview(scratch, PADW)
            d = cview(dst)
            V.tensor_tensor(b[:, :, 0:525], a[:, :, 0:525], a[:, :, 1:526], Alu.min)
            V.tensor_tensor(a[:, :, 0:523], b[:, :, 0:523], b[:, :, 2:525], Alu.min)
            V.tensor_tensor(b[:, :, 0:519], a[:, :, 0:519], a[:, :, 4:523], Alu.min)
            V.tensor_tensor(d[:, 0:NCHUNK, :], b[:, :, 0:512], b[:, :, 7:519],
                            Alu.min)

        def transpose_blocks(dst_ap, src_flat):
            sv = cview(src_flat)
            pt = pbig.tile([128, NW], bf16, name="pt", tag="ptp")
            for co in range(NCHUNK):
                for ci in range(NCHUNK):
                    nc.tensor.transpose(
                        pt[:, co * CW + ci * 128: co * CW + (ci + 1) * 128],
                        sv[:, ci, co * 128:(co + 1) * 128], cbf["ident"][:])
            nc.scalar.copy(dst_ap, cview(pt)[:, :, :])

        def t_fwd(s):
            memset_pads(uhTp, G)
            iv = cview(uhTp, PADW)
            transpose_blocks(iv[:, :, WIN_PAD:WIN_PAD + CW], uh[s])

        def t_back(s):
            transpose_blocks(cview(uh[s])[:, :, :], poolT)

        # ------------------------------------------------ sub-grid helpers
        def pe_sub(dst_seg_ap, src_full, scale=1.0, bias=0.0):
            """dst (sub scan-layout data view) <- src_full[::2,::2]*scale+bias."""
            sv = cview(src_full)
            ps = pmid.tile([128, NWS], f32, name="subps", tag="pmid")
            for cs in range(NCS):
                psc = ps[:, cs * WS:(cs + 1) * WS]
                nc.tensor.matmul(psc, cbf["selA"][:],
                                 sv[:, 2 * cs, 0:CW:2], start=True, stop=False)
                nc.tensor.matmul(psc, cbf["selB"][:],
                                 sv[:, 2 * cs + 1, 0:CW:2], start=False,
                                 stop=True)
            if scale == 1.0 and bias == 0.0:
                nc.scalar.copy(dst_seg_ap, cview(ps, WS)[:, :, :])
            else:
                nc.scalar.activation(dst_seg_ap, cview(ps, WS)[:, :, :],
                                     Act.Copy, bias=bias, scale=scale)

        def pe_sub_T(dst_seg_ap, scale, bias):
            """subsample from poolT (transposed pooled image) + re-transpose:
            avoids the full-res back-transpose of the dark2 pool."""
            pv = cview(poolT)
            ps = pmid.tile([128, NWS], f32, name="subTps", tag="pmid")
            for cs in range(NCS):
                psc = ps[:, cs * WS:(cs + 1) * WS]
                nc.tensor.matmul(psc, cbf["selA"][:],
                                 pv[:, 2 * cs, 0:CW:2], start=True, stop=False)
                nc.tensor.matmul(psc, cbf["selB"][:],
                                 pv[:, 2 * cs + 1, 0:CW:2], start=False,
                                 stop=True)
            tT = dout.tile([128, NWS], bf16, name="tT", tag="dout")
            nc.scalar.copy(tT[:], ps[:])
            ps2 = pmid.tile([128, NWS], bf16, name="subT2", tag="pmid")
            tv = cview(tT, WS)
            for ch in range(NCS):
                for cw in range(NCS):
                    nc.tensor.transpose(
                        ps2[:, ch * WS + cw * 128: ch * WS + (cw + 1) * 128],
                        tv[:, cw, ch * 128:(ch + 1) * 128], cbf["ident"][:])
            nc.scalar.activation(dst_seg_ap, cview(ps2, WS)[:, :, :],
                                 Act.Copy, bias=bias, scale=scale)

        def hbox_s(hb_t, src_t):
            cum = cump.tile([128, SCN_W], f32, name="cum", tag="cum")
            for c in range(NCS):
                V.tensor_tensor_scan(cum[:, c * SEG:(c + 1) * SEG],
                                     src_t[:, c * SEG:(c + 1) * SEG],
                                     fbcast(c_ones128[:, 0:1], SEG), 0.0,
                                     Alu.add, Alu.bypass)
            V.tensor_tensor(cview(hb_t, WS)[:, :, :],
                            segview(cum, LEAD + RS),
                            segview(cum, LEAD - RS - 1), Alu.subtract)

        def vbox_s(dst, src):
            sv = cview(src, WS)
            ps = pmid.tile([128, NWS], f32, name="vps", tag="pmid")
            r0 = ps[:, 0:WS]
            r1 = ps[:, WS:NWS]
            nc.tensor.matmul(r0, cbf["bm0s"][:], sv[:, 0, :], start=True,
                             stop=False)
            nc.tensor.matmul(r0, cbf["bds"][:], sv[:, 1, :], start=False,
                             stop=True)
            nc.tensor.matmul(r1, cbf["bm1s"][:], sv[:, 1, :], start=True,
                             stop=False)
            nc.tensor.matmul(r1, cbf["bus"][:], sv[:, 0, :], start=False,
                             stop=True)
            nc.scalar.copy(dst[:], ps[:])
            db = dst[:]
            edges = bass.AP(tensor=db.tensor, offset=db.offset,
                            ap=[list(db.ap[0]), [WS, NCS],
                                [WS - RS, 2], [1, RS]])
            fb = c_fixs[:]
            fv = bass.AP(tensor=fb.tensor, offset=fb.offset,
                         ap=[list(fb.ap[0]), [2 * RS, NCS], [RS, 2], [1, RS]])
            V.tensor_tensor(edges, edges, fv, Alu.mult)

        def upsample(dst_full_bf16, src_sub):
            """bilinear 2x upsample [128, 2x256] f32 -> [128, 4x512] bf16."""
            wide = mrot.tile([128, NCS * CW], bf16, name="wide", tag="wide",
                             bufs=2)
            wv = cview(wide)
            sv = cview(src_sub, WS)
            # W-upsample at sub rows
            wide_e = bass.AP(tensor=wv.tensor, offset=wv.offset,
                             ap=[list(wv.ap[0]), [CW, NCS], [2, WS]])
            V.tensor_copy(wide_e, sv[:, :, :])
            wide_o = bass.AP(tensor=wv.tensor, offset=wv.offset + 1,
                             ap=[list(wv.ap[0]), [CW, NCS], [2, WS - 1]])
            V.tensor_tensor(wide_o, sv[:, :, 0:WS - 1], sv[:, :, 1:WS], Alu.add)
            V.tensor_scalar(wide_o, wide_o, 0.5, 0.0, Alu.mult, Alu.add)
            lastc = bass.AP(tensor=wv.tensor, offset=wv.offset + CW - 1,
                            ap=[list(wv.ap[0]), [CW, NCS], [1, 1]])
            V.tensor_copy(lastc, sv[:, :, WS - 1:WS])
            # H-upsample via PE
            ps = pbig.tile([128, NW], f32, name="ups", tag="ptp")
            for c, mats in enumerate((
                    (("u00", 0),), (("u10", 0), ("u11", 1)),
                    (("u21", 1),), (("u31", 1),))):
                psc = ps[:, c * CW:(c + 1) * CW]
                for i, (nm, sc) in enumerate(mats):
                    nc.tensor.matmul(psc, cbf[nm][:], wv[:, sc, :],
                                     start=(i == 0), stop=(i == len(mats) - 1))
            nc.scalar.copy(dst_full_bf16[:], ps[:])

        # ---------------------------------------------- per-sample frontend
        ST = [dict(), dict()]

        def f_load(s):
            for chn in range(3):
                src = x_ext[s, chn].rearrange("(c p) w -> p c w", p=128)
                if s == 0:
                    # sample 0 via HWDGE f32 + ACT cast: full-rate load off the
                    # (slower, serialized) SWDGE cast queue
                    stg = abt.tile([128, NW], f32, name=f"xstg{chn}",
                                   tag="abt")
                    nc.sync.dma_start(out=cview(stg)[:, :, :], in_=src)
                    nc.scalar.copy(x16[s][chn][:], stg[:])
                else:
                    nc.gpsimd.dma_start(out=cview(x16[s][chn])[:, :, :], in_=src)

        def f_guid(s):
            gt = t_guid[s]
            tg = dout.tile([128, NW], bf16, name=f"gt{s}", tag="dout")
            tb = dout.tile([128, NW], bf16, name=f"bt{s}", tag="dout")
            nc.scalar.activation(gt[:], x16[s][0][:], Act.Copy,
                                 bias=0.5, scale=0.14945)
            nc.scalar.activation(tg[:], x16[s][1][:], Act.Copy,
                                 bias=0.0, scale=0.2935)
            nc.scalar.activation(tb[:], x16[s][2][:], Act.Copy,
                                 bias=0.0, scale=0.057)
            V.tensor_tensor(gt[:], gt[:], tg[:], Alu.add)
            V.tensor_tensor(gt[:], gt[:], tb[:], Alu.add)

        def f_chanmin_hpool(s, second):
            memset_pads(mxp, G)
            if not second:
                a0, a1, a2 = x16[s]
                V.tensor_tensor(interior(mxp), cview(a0)[:, :, :],
                                cview(a1)[:, :, :], Alu.min)
                V.tensor_tensor(interior(mxp), interior(mxp),
                                cview(a2)[:, :, :], Alu.min)
            else:
                chsc = ST[s]["chsc"]
                ytmp = junk
                nc.scalar.activation(interior(mxp), x16[s][0][:], Act.Identity,
                                     bias=chsc[:, 3:4], scale=chsc[:, 3:4])
                nc.scalar.activation(ytmp, x16[s][1][:], Act.Identity,
                                     bias=chsc[:, 4:5], scale=chsc[:, 4:5])
                V.tensor_tensor(interior(mxp), interior(mxp), junk_c, Alu.min)
                nc.scalar.activation(ytmp, x16[s][2][:], Act.Identity,
                                     bias=chsc[:, 5:6], scale=chsc[:, 5:6])
                V.tensor_tensor(interior(mxp), interior(mxp), junk_c, Alu.min)
            hpool(uh[s], mxp, w1)

        def f_hpoolT(s):
            hpool(poolT, uhTp, w1)

        def dark_phase(second):
            f_chanmin_hpool(0, second)
            t_fwd(0)
            f_chanmin_hpool(1, second)
            if not second:
                f_guid(0)
            f_hpoolT(0)
            t_back(0)
            t_fwd(1)
            if not second:
                f_guid(1)
            f_hpoolT(1)
            t_back(1)

        # ------------------------------------------------------- secant/topk
        def f_secant_init(s):
            st = ST[s]
            st["acc8"] = tiny.tile([128, 8], f32, name=f"acc8{s}", tag=f"acc8{s}")
            V.memset(st["acc8"][:], 0.0)
            st["thr"] = tiny.tile([128, 1], f32, name=f"thr{s}", tag=f"thr{s}")
            st["scal"] = tiny.tile([1, 16], f32, name=f"scal{s}", tag=f"scal{s}")
            V.memset(st["scal"][:], 0.0)
            V.memset(st["scal"][:, 0:1], T0)
            V.memset(st["scal"][:, 2:3], T1)

        def count_into(s, col, sub=False):
            st = ST[s]
            u, acc8, thr = uh[s], st["acc8"], st["thr"]
            uv = cview(u)
            if sub:
                V.tensor_scalar(junk_c[:, 0:2, 0:256],
                                uv[:, 0:NCHUNK:2, 0:CW:2], thr[:], 0.0,
                                Alu.is_gt, Alu.add,
                                accum_out=acc8[:, col:col + 1])
            else:
                V.tensor_scalar(junk, u[:, 0:NW], thr[:], 0.0,
                                Alu.is_gt, Alu.add,
                                accum_out=acc8[:, col:col + 1])
            fps = psml.tile([1, 1], f32, name=f"fold{s}", tag=f"fold{s}")
            nc.tensor.matmul(fps[:], c_ones128[:], acc8[:, col:col + 1],
                             start=True, stop=True)
            return fps

        def bcast_thr(s, src_col):
            st = ST[s]
            bp = psml.tile([128, 1], f32, name=f"thrps{s}", tag=f"fold{s}")
            nc.tensor.matmul(bp[:], c_ones1x[:], src_col, start=True, stop=True)
            nc.scalar.copy(st["thr"][:], bp[:])

        def f_count0(s, which):
            scal = ST[s]["scal"]
            col = 0 if which == 0 else 2
            bcast_thr(s, scal[0:1, col:col + 1])
            f = count_into(s, 0, sub=True)
            nc.scalar.copy(scal[:, col + 1:col + 2], f[:])

        def f_secant_round(s, rnd):
            scal = ST[s]["scal"]
            full = rnd >= SECANT_ROUNDS - 2
            if rnd == SECANT_ROUNDS - 2:
                V.tensor_scalar(scal[:, 1:2], scal[:, 1:2], 4.0, 0.0,
                                Alu.mult, Alu.add)
                V.tensor_scalar(scal[:, 3:4], scal[:, 3:4], 4.0, 0.0,
                                Alu.mult, Alu.add)
            V.tensor_tensor(scal[:, 4:5], scal[:, 3:4], scal[:, 1:2], Alu.subtract)
            V.tensor_scalar(scal[:, 8:9], scal[:, 4:5], -1.0, 0.0, Alu.mult, Alu.add)
            V.tensor_tensor(scal[:, 4:5], scal[:, 4:5], scal[:, 8:9], Alu.max)
            V.tensor_scalar(scal[:, 4:5], scal[:, 4:5], 1.0, 0.0, Alu.max, Alu.add)
            V.tensor_tensor(scal[:, 5:6], scal[:, 2:3], scal[:, 0:1], Alu.subtract)
            V.tensor_scalar(scal[:, 8:9], scal[:, 5:6], -1.0, 0.0, Alu.mult, Alu.add)
            V.tensor_tensor(scal[:, 5:6], scal[:, 5:6], scal[:, 8:9], Alu.max)
            V.reciprocal(scal[:, 8:9], scal[:, 4:5])
            V.tensor_tensor(scal[:, 5:6], scal[:, 5:6], scal[:, 8:9], Alu.mult)
            V.tensor_scalar(scal[:, 6:7], scal[:, 3:4], 1.0,
                            -float(TOPN) if full else -TOPN / 4.0,
                            Alu.mult, Alu.add)
            V.tensor_tensor(scal[:, 6:7], scal[:, 6:7], scal[:, 5:6], Alu.mult)
            V.tensor_copy(scal[:, 0:1], scal[:, 2:3])
            V.tensor_copy(scal[:, 1:2], scal[:, 3:4])
            V.tensor_tensor(scal[:, 2:3], scal[:, 2:3], scal[:, 6:7], Alu.add)
            bcast_thr(s, scal[0:1, 2:3])
            f = count_into(s, 0, sub=not full)
            nc.scalar.copy(scal[:, 3:4], f[:])

        def f_msums(s):
            st = ST[s]
            u, acc8, thr = uh[s], st["acc8"], st["thr"]
            V.tensor_scalar(junk, u[:, 0:NW], thr[:], 0.0,
                            Alu.is_gt, Alu.bypass)
            mbufs = (poolT[:], uhTp[:, 0:NW], mxp[:, 0:NW])
            for chn, xt in enumerate(x16[s]):
                mb = mbufs[chn % 3]
                V.tensor_tensor(mb, junk, xt[:], Alu.mult)
                nc.scalar.activation(mb, mb, Act.Copy,
                                     accum_out=acc8[:, 1 + chn:2 + chn])

        def f_bandprep(s):
            st = ST[s]
            scal = st["scal"]
            V.tensor_scalar(scal[:, 7:8], scal[:, 2:3], 1.0, -BAND,
                            Alu.mult, Alu.add)
            bcast_thr(s, scal[0:1, 7:8])

        def f_bandsums(s):
            st = ST[s]
            u, acc8, thr = uh[s], st["acc8"], st["thr"]
            V.tensor_scalar(junk, u[:, 0:NW], thr[:], 0.0,
                            Alu.is_gt, Alu.bypass)
            nc.scalar.activation(poolT[:], junk, Act.Copy,
                                 accum_out=acc8[:, 4:5])
            mbufs = (poolT[:], uhTp[:, 0:NW], mxp[:, 0:NW])
            for chn, xt in enumerate(x16[s]):
                mb = mbufs[chn % 3]
                V.tensor_tensor(mb, junk, xt[:], Alu.mult)
                nc.scalar.activation(mb, mb, Act.Copy,
                                     accum_out=acc8[:, 5 + chn:6 + chn])

        def f_afold(s):
            st = ST[s]
            tps = psml.tile([1, 8], f32, name=f"totps{s}", tag=f"fold{s}")
            nc.tensor.matmul(tps[:], c_ones128[:], st["acc8"][:],
                             start=True, stop=True)
            tot = tiny.tile([1, 8], f32, name=f"tot{s}", tag=f"tot{s}")
            nc.scalar.copy(tot[:], tps[:])
            st["tot"] = tot

        def f_amath(s):
            st = ST[s]
            tot = st["tot"]
            am = tiny.tile([1, 12], f32, name=f"am{s}", tag=f"am{s}")
            V.tensor_tensor(am[:, 0:3], tot[:, 5:8], tot[:, 1:4], Alu.subtract)
            V.tensor_tensor(am[:, 11:12], tot[:, 4:5], tot[:, 0:1], Alu.subtract)
            V.tensor_scalar(am[:, 11:12], am[:, 11:12], 1.0, 0.0, Alu.max, Alu.add)
            V.reciprocal(am[:, 10:11], am[:, 11:12])
            V.tensor_tensor(am[:, 0:3], am[:, 0:3], fbcast(am[:, 10:11], 3), Alu.mult)
            V.tensor_scalar(am[:, 9:10], tot[:, 0:1], -1.0, float(TOPN),
                            Alu.mult, Alu.add)
            V.tensor_tensor(am[:, 0:3], am[:, 0:3], fbcast(am[:, 9:10], 3), Alu.mult)
            V.tensor_tensor(am[:, 0:3], am[:, 0:3], tot[:, 1:4], Alu.add)
            V.tensor_scalar(am[:, 0:3], am[:, 0:3], 1.0 / TOPN, 0.0, Alu.mult, Alu.add)
            V.tensor_scalar(am[:, 3:6], am[:, 0:3], 1.0, 1.0, Alu.mult, Alu.add)
            V.reciprocal(am[:, 3:6], am[:, 3:6])
            V.tensor_scalar(am[:, 0:3], am[:, 0:3], 0.5, 0.5, Alu.mult, Alu.add)
            V.tensor_scalar(am[:, 6:9], am[:, 0:3], -1.0, 0.5, Alu.mult, Alu.add)
            st["am"] = am

        def f_chsc(s):
            st = ST[s]
            st["chsc"] = tiny.tile([128, 9], f32, name=f"chsc{s}",
                                   tag=f"chsc{s}")
            bp = psml.tile([128, 9], f32, name=f"chps{s}", tag=f"fold{s}")
            nc.tensor.matmul(bp[:], c_ones1x[:], st["am"][0:1, 0:9],
                             start=True, stop=True)
            nc.scalar.copy(st["chsc"][:], bp[:])

        # ------------------------------------------- guidance-only box prep
        def prep_ops(s):
            yield lambda: pe_sub(sview(t_is[s]), t_guid[s])
            yield lambda: nc.scalar.activation(sview(t_ii[s]), sview(t_is[s]),
                                               Act.Square)
            hbI = [None]
            hbII = [None]
            mII = [None]

            def scanI():
                hbI[0] = rot.tile([128, NWS], f32r, name="hbI", tag="hbx")
                hbox_s(hbI[0], t_is[s])
            yield scanI
            yield lambda: vbox_s(mean_Is[s], hbI[0])

            def scanII():
                hbII[0] = rot.tile([128, NWS], f32r, name="hbII", tag="hbx")
                hbox_s(hbII[0], t_ii[s])
            yield scanII

            def vboxII():
                mII[0] = mrot.tile([128, NWS], f32, name="mII", tag="mpx")
                vbox_s(mII[0], hbII[0])
            yield vboxII

            def varrec():
                sq = sab.tile([128, NWS], f32, name="sq", tag="sab")
                nc.scalar.activation(sq[:], mean_Is[s][:], Act.Square)
                V.scalar_tensor_tensor(sq[:], mII[0][:], EPS, sq[:],
                                       Alu.add, Alu.subtract)
                V.reciprocal_approx_fast(out=rec_s[s][:], in_=sq[:])
            yield varrec

        # ---------------------------------------------------------- backend
        BK = [dict(), dict()]

        def backend_head(s):
            pe_sub(sview(t_ps[s]), uh[s], scale=-OMEGA, bias=1.0)
            V.tensor_tensor(sview(t_ip[s]), sview(t_is[s]), sview(t_ps[s]),
                            Alu.mult)
            hb_p = rot.tile([128, NWS], f32r, name="hb_p", tag="hbx")
            hbox_s(hb_p, t_ps[s])
            mean_p = mrot.tile([128, NWS], f32, name="mean_p", tag="mpx")
            vbox_s(mean_p, hb_p)
            hb_ip = rot.tile([128, NWS], f32r, name="hb_ip", tag="hbx")
            hbox_s(hb_ip, t_ip[s])
            mean_Ip = mrot.tile([128, NWS], f32, name="mean_Ip", tag="mpx")
            vbox_s(mean_Ip, hb_ip)
            BK[s]["mp"], BK[s]["mip"] = mean_p, mean_Ip

        def backend_mid(s):
            mean_p, mean_Ip = BK[s]["mp"], BK[s]["mip"]
            tmp = sab.tile([128, NWS], f32, name="tmp", tag="sab")
            V.tensor_tensor(tmp[:], mean_Is[s][:], mean_p[:], Alu.mult)
            cov = sab.tile([128, NWS], f32, name="cov", tag="sab")
            V.tensor_tensor(cov[:], mean_Ip[:], tmp[:], Alu.subtract)
            a_v = sview(t_ip[s])          # overwrite Ip (dead) with a
            V.tensor_tensor(a_v, cview(cov, WS)[:, :, :],
                            cview(rec_s[s], WS)[:, :, :], Alu.mult)
            t2 = sab.tile([128, NWS], f32, name="t2", tag="sab")
            V.tensor_tensor(cview(t2, WS)[:, :, :], a_v,
                            cview(mean_Is[s], WS)[:, :, :], Alu.mult)
            b_v = sview(t_ps[s])          # overwrite p (dead) with b
            V.tensor_tensor(b_v, cview(mean_p, WS)[:, :, :],
                            cview(t2, WS)[:, :, :], Alu.subtract)

            hba = rot.tile([128, NWS], f32r, name="hba", tag="hbx")
            hbox_s(hba, t_ip[s])
            mean_a = mrot.tile([128, NWS], f32, name="mean_a", tag="mpx")
            vbox_s(mean_a, hba)
            hbb = rot.tile([128, NWS], f32r, name="hbb", tag="hbx")
            hbox_s(hbb, t_ps[s])
            mean_b = mrot.tile([128, NWS], f32, name="mean_b", tag="mpx")
            vbox_s(mean_b, hbb)
            ma_f = mfull.tile([128, NW], bf16, name="ma_f", tag="mf")
            upsample(ma_f, mean_a)
            mb_f = mfull.tile([128, NW], bf16, name="mb_f", tag="mf")
            upsample(mb_f, mean_b)
            BK[s]["ma"], BK[s]["mb"] = ma_f, mb_f

        def backend_tail(s):
            chsc = ST[s]["chsc"]
            ma_f, mb_f = BK[s]["ma"], BK[s]["mb"]
            HW_ = NW // 2
            T16 = dout.tile([128, NW], bf16, name="T16", tag="dout")
            T_t = abt.tile([128, NW], f32, name="T_t", tag="abt")
            rT = abt.tile([128, NW], f32, name="rT", tag="abt")
            for h in (0, 1):
                sl = slice(h * HW_, (h + 1) * HW_)
                V.tensor_tensor(T16[:, sl], ma_f[:, sl], t_guid[s][:, sl],
                                Alu.mult)
                V.tensor_tensor(T16[:, sl], T16[:, sl], mb_f[:, sl], Alu.add)
                nc.scalar.copy(T_t[:, sl], T16[:, sl])
                V.reciprocal_approx_fast(out=rT[:, sl], in_=T_t[:, sl])
                if s == 0:
                    nc.scalar.copy(poolT[:, sl], rT[:, sl])
            rmul = poolT if s == 0 else rT

            for chn in range(3):
                d_t = dout.tile([128, NW], bf16, name=f"d{chn}", tag="dout")
                yv = y_ext[s, chn].rearrange("(c p) w -> p c w", p=128)
                for h in (0, 1):
                    sl = slice(h * HW_, (h + 1) * HW_)
                    nc.scalar.activation(d_t[:, sl], x16[s][chn][:, sl],
                                         Act.Identity,
                                         bias=chsc[:, 6 + chn:7 + chn],
                                         scale=0.5)
                    V.tensor_tensor(d_t[:, sl], d_t[:, sl], rmul[:, sl],
                                    Alu.mult)
                    V.tensor_scalar(d_t[:, sl], d_t[:, sl],
                                    chsc[:, chn:chn + 1], 0.0,
                                    Alu.add, Alu.add)
                    nc.gpsimd.dma_start(out=yv[:, 2 * h:2 * h + 2, :],
                                        in_=cview(d_t)[:, 2 * h:2 * h + 2, :])

        # ================================================== emission order
        f_load(0)
        f_load(1)
        load_consts()
        dark_phase(second=False)

        f_secant_init(0)
        f_secant_init(1)
        preps = list(prep_ops(0)) + list(prep_ops(1))
        pi = 0

        def drain_prep(n=1):
            nonlocal pi
            for _ in range(n):
                if pi < len(preps):
                    preps[pi]()
                    pi += 1

        for which in (0, 1):
            f_count0(0, which)
            drain_prep()
            f_count0(1, which)
            drain_prep()
        for rnd in range(SECANT_ROUNDS):
            f_secant_round(0, rnd)
            drain_prep()
            f_secant_round(1, rnd)
            drain_prep()
        f_msums(0)
        f_msums(1)
        f_bandprep(0)
        f_bandprep(1)
        drain_prep(2)
        f_bandsums(0)
        f_bandsums(1)
        drain_prep(len(preps))
        f_afold(0)
        f_afold(1)
        f_amath(0)
        f_amath(1)
        f_chsc(0)
        f_chsc(1)

        # dark2 phase with backend_head(0) interleaved after sample 0's pool
        f_chanmin_hpool(0, True)
        t_fwd(0)
        f_chanmin_hpool(1, True)
        f_hpoolT(0)
        t_back(0)
        backend_head(0)
        t_fwd(1)
        f_hpoolT(1)
        t_back(1)
        backend_mid(0)
        backend_head(1)
        backend_tail(0)
        backend_mid(1)
        backend_tail(1)

    nc.compile()
    return nc


def _get_program():
    if "nc" not in _CACHE:
        _CACHE["nc"] = _build()
    return _CACHE["nc"]


def kernel(x: np.ndarray) -> np.ndarray:
    from concourse.bass_utils import run_bass_kernel_spmd
    x = np.ascontiguousarray(np.asarray(x, dtype=np.float32))
    assert x.shape == (16, 3, H, W), x.shape
    nc = _get_program()
    consts = _host_consts()
    in_maps = [{"x": x[2 * i:2 * i + 2], **consts} for i in range(8)]
    res = run_bass_kernel_spmd(nc, in_maps, list(range(8)))
    out = np.concatenate([res.results[i]["y"] for i in range(8)], axis=0)
    return out.astype(np.float32)


# revision 59
# speedup vs baseline: 1.0254x; 1.0254x over previous
"""Dark-Channel-Prior dehazing (DCPGenerator) Trainium2 Bass kernel, v9.

v8 -> v9: the guided filter runs as a fast-guided-filter at 2x subsample
(256x256): all six box filters (I, II, p, Ip, a, b), the cov/var/a/b
math, and the vbox matmuls operate on 1/4 the pixels with radius-20
bands; mean_a/mean_b are bilinearly upsampled (PE matmuls for rows, DVE
for columns) and T = mean_a*I + mean_b is applied at full resolution.
Subsampling of guid / pooled-dark runs on the PE with selection
matrices.  Dark channel, top-k secant, and A estimation stay full-res.
"""
import numpy as np
from contextlib import ExitStack

H = 512
W = 512
NCHUNK = 4
CW = 512
NW = NCHUNK * CW            # 2048
PADW = 526                  # 7 | 512 | 7
WIN_PAD = 7
RADIUS = 40
# sub-grid (fast guided filter, s=2)
HS = 256
WS = 256
NCS = 2
RS = 20
LEAD = 24                   # leading zeros in sub scan layout (>=RS+1)
SEG = 300                   # WS + 44-zero gap (>= 2*RS+1)
SCN_W = LEAD + NCS * SEG    # 624
NWS = NCS * WS              # 512
EPS = 1e-3
OMEGA = 0.95
TOPN = int(0.01 * H * W)    # 2621
T0 = 0.0055
T1 = 0.0085
BAND = 2e-4
SECANT_ROUNDS = 5

_CACHE = {}


def _host_consts():
    # full-res H-direction box weights are no longer needed; sub-grid ones:
    i = np.arange(HS)
    n1s = np.minimum(i + RS, HS - 1) - np.maximum(i - RS, 0) + 1
    inv_ns = (1.0 / n1s).astype(np.float32)
    k = np.arange(128)[:, None]
    p = np.arange(128)[None, :]
    bands = (np.abs(k - p) <= RS).astype(np.float32)
    bus = (k >= p + 128 - RS).astype(np.float32) / 41.0 / 41.0
    bds = (k <= p - (128 - RS)).astype(np.float32) / 41.0 / 41.0
    bm0s = bands * inv_ns[0:128][None, :] / 41.0
    bm1s = bands * inv_ns[128:256][None, :] / 41.0
    fix40 = np.concatenate([41.0 * inv_ns[0:RS], 41.0 * inv_ns[WS - RS:]])
    fixs = np.tile(fix40[None, :], (128, NCS)).copy()        # [128, 80]
    ident = np.eye(128, dtype=np.float32)
    # row-subsample selection: out q <- full partition 2q (two half matrices)
    selA = np.zeros((128, 128), np.float32)
    selB = np.zeros((128, 128), np.float32)
    for q in range(64):
        selA[2 * q, q] = 1.0
    for q in range(64, 128):
        selB[2 * (q - 64), q] = 1.0
    # row-upsample (bilinear, sub sample i at full row 2i)
    U = {}
    for c in range(NCHUNK):
        for q in range(128):
            r = 128 * c + q
            if r % 2 == 0:
                pairs = [(r // 2, 1.0)]
            else:
                i0 = (r - 1) // 2
                i1 = min(i0 + 1, HS - 1)
                pairs = [(i0, 0.5), (i1, 0.5)] if i1 != i0 else [(i0, 1.0)]
            for i_, wgt in pairs:
                sc, pp_ = divmod(i_, 128)
                U.setdefault((c, sc), np.zeros((128, 128), np.float32))[
                    pp_, q] += wgt
    return {"bm0s": bm0s, "bm1s": bm1s, "bus": bus, "bds": bds,
            "fixs": fixs, "ident": ident, "selA": selA, "selB": selB,
            "u00": U[(0, 0)], "u10": U[(1, 0)], "u11": U[(1, 1)],
            "u21": U[(2, 1)], "u31": U[(3, 1)]}


def _build():
    import concourse.bacc as bacc
    import concourse.tile as tile
    import concourse.bass as bass
    from concourse import mybir

    f32 = mybir.dt.float32
    f32r = mybir.dt.float32r
    bf16 = mybir.dt.bfloat16
    Alu = mybir.AluOpType
    Act = mybir.ActivationFunctionType

    nc = bacc.Bacc("TRN2", target_bir_lowering=False, debug=False, num_devices=8)
    V = nc.vector
    G = nc.gpsimd

    x_ext = nc.dram_tensor("x", [2, 3, H, W], f32, kind="ExternalInput").ap()
    c128_names = ("bm0s", "bm1s", "bus", "bds", "ident", "selA", "selB",
                  "u00", "u10", "u11", "u21", "u31")
    c128_exts = {nm: nc.dram_tensor(nm, [128, 128], f32, kind="ExternalInput").ap()
                 for nm in c128_names}
    fixs_ext = nc.dram_tensor("fixs", [128, NCS * 2 * RS], f32,
                              kind="ExternalInput").ap()
    y_ext = nc.dram_tensor("y", [2, 3, H, W], f32, kind="ExternalOutput").ap()

    def cview(t, width=CW):
        return t.rearrange("p (c w) -> p c w", w=width)

    def fbcast(ap_col, n):
        return bass.AP(tensor=ap_col.tensor, offset=ap_col.offset,
                       ap=[list(p) for p in ap_col.ap[:-1]] + [[0, n]])

    def segview(t, off, c0=0, nch=NCS):
        """[128, nch, WS] view into a [128, SCN_W] sub tile."""
        base = t[:]
        return bass.AP(tensor=base.tensor, offset=base.offset + off + c0 * SEG,
                       ap=[list(base.ap[0]), [SEG, nch], [1, WS]])

    def sview(t):
        return segview(t, LEAD)

    with ExitStack() as ctx:
        tc = ctx.enter_context(tile.TileContext(nc))

        cpool = ctx.enter_context(tc.tile_pool(name="cpool", bufs=1))
        srcp = ctx.enter_context(tc.tile_pool(name="srcp", bufs=1))
        scn = ctx.enter_context(tc.tile_pool(name="scn", bufs=1))
        pp = ctx.enter_context(tc.tile_pool(name="pp", bufs=1))
        cump = ctx.enter_context(tc.tile_pool(name="cump", bufs=2))
        boxes = ctx.enter_context(tc.tile_pool(name="boxes", bufs=1))
        rot = ctx.enter_context(tc.tile_pool(name="rot", bufs=2))
        mrot = ctx.enter_context(tc.tile_pool(name="mrot", bufs=4))
        abt = ctx.enter_context(tc.tile_pool(name="abt", bufs=3))
        sab = ctx.enter_context(tc.tile_pool(name="sab", bufs=3))
        dout = ctx.enter_context(tc.tile_pool(name="dout", bufs=2))
        mfull = ctx.enter_context(tc.tile_pool(name="mfull", bufs=2))
        tiny = ctx.enter_context(tc.tile_pool(name="tiny", bufs=1))
        pbig = ctx.enter_context(tc.tile_pool(name="pbig", bufs=1, space="PSUM"))
        pmid = ctx.enter_context(tc.tile_pool(name="pmid", bufs=2, space="PSUM"))
        psml = ctx.enter_context(tc.tile_pool(name="psml", bufs=1, space="PSUM"))

        # ------------------------------------- constants (loaded after x DMAs)
        cbf = {}
        stage = cpool.tile([128, 128], f32, name="s_band")
        for nm in ("bm0s", "bm1s", "bus", "bds"):
            cbf[nm] = cpool.tile([128, 128], f32r, name=f"c_{nm}")
        for nm in ("ident", "selA", "selB", "u00", "u10", "u11", "u21", "u31"):
            cbf[nm] = cpool.tile([128, 128], bf16, name=f"c_{nm}")
        c_fixs = cpool.tile([128, NCS * 2 * RS], f32, name="c_fixs")
        c_ones128 = cpool.tile([128, 1], f32, name="c_ones128")
        c_ones1x = cpool.tile([1, 128], f32, name="c_ones1x")

        def load_consts():
            for nm in ("bm0s", "bm1s", "bus", "bds", "ident", "selA", "selB",
                       "u00", "u10", "u11", "u21", "u31"):
                nc.sync.dma_start(out=stage[:], in_=c128_exts[nm][:])
                nc.scalar.copy(cbf[nm][:], stage[:])
            nc.sync.dma_start(out=c_fixs[:], in_=fixs_ext[:])
            V.memset(c_ones128[:], 1.0)
            V.memset(c_ones1x[:], 1.0)

        # --------------------------------------------------- persistent tiles
        x16 = [[srcp.tile([128, NW], bf16, name=f"x16_{s}_{c}")
                for c in range(3)] for s in range(2)]
        t_guid = [srcp.tile([128, NW], bf16, name=f"guid{s}") for s in range(2)]
        # sub-grid scan-layout sources (f32): I, p, Ip, II, a, b per sample
        t_is = [scn.tile([128, SCN_W], f32, name=f"is{s}") for s in range(2)]
        t_ps = [scn.tile([128, SCN_W], f32, name=f"ps{s}") for s in range(2)]
        t_ip = [scn.tile([128, SCN_W], f32, name=f"ip{s}") for s in range(2)]
        t_ii = [scn.tile([128, SCN_W], f32, name=f"ii{s}") for s in range(2)]
        mxp = pp.tile([128, NCHUNK * PADW], bf16, name="mxp")
        w1 = pp.tile([128, NCHUNK * PADW], bf16, name="w1")
        uhTp = pp.tile([128, NCHUNK * PADW], bf16, name="uhTp")
        poolT = pp.tile([128, NW], bf16, name="poolT")
        uh = [pp.tile([128, NW], bf16, name=f"uh{s}") for s in range(2)]
        mean_Is = [boxes.tile([128, NWS], f32, name=f"meanIs{s}")
                   for s in range(2)]
        rec_s = [boxes.tile([128, NWS], f32, name=f"recs{s}") for s in range(2)]

        junk = w1[:, 0:NW]
        junk_c = junk.rearrange("p (c w) -> p c w", w=CW)

        # zero the sub scan-layout gaps once
        for t in (t_is[0], t_is[1], t_ps[0], t_ps[1], t_ip[0], t_ip[1],
                  t_ii[0], t_ii[1]):
            V.memset(t[:, 0:LEAD], 0.0)
            for c in range(NCS):
                V.memset(t[:, LEAD + c * SEG + WS: LEAD + (c + 1) * SEG], 0.0)

        # ---------------------------------------------------------- helpers
        def interior(t):
            return cview(t, PADW)[:, :, WIN_PAD:WIN_PAD + CW]

        def memset_pads(t, eng):
            v = cview(t, PADW)
            for c in range(NCHUNK):
                eng.memset(v[:, c, 0:WIN_PAD], 1.0)
                eng.memset(v[:, c, PADW - WIN_PAD:PADW], 1.0)

        def hpool(dst, padded, scratch):
            a = cview(padded, PADW)
            b = cview(scratch, PADW)
            d = cview(dst)
            V.tensor_tensor(b[:, :, 0:525], a[:, :, 0:525], a[:, :, 1:526], Alu.min)
            V.tensor_tensor(a[:, :, 0:523], b[:, :, 0:523], b[:, :, 2:525], Alu.min)
            V.tensor_tensor(b[:, :, 0:519], a[:, :, 0:519], a[:, :, 4:523], Alu.min)
            V.tensor_tensor(d[:, 0:NCHUNK, :], b[:, :, 0:512], b[:, :, 7:519],
                            Alu.min)

        def transpose_blocks(dst_ap, src_flat, on_dve=False):
            sv = cview(src_flat)
            pt = pbig.tile([128, NW], bf16, name="pt", tag="ptp")
            for co in range(NCHUNK):
                for ci in range(NCHUNK):
                    nc.tensor.transpose(
                        pt[:, co * CW + ci * 128: co * CW + (ci + 1) * 128],
                        sv[:, ci, co * 128:(co + 1) * 128], cbf["ident"][:])
            if on_dve:
                V.tensor_copy(dst_ap, cview(pt)[:, :, :])
            else:
                nc.scalar.copy(dst_ap, cview(pt)[:, :, :])

        def t_fwd(s, on_dve=False):
            memset_pads(uhTp, G)
            iv = cview(uhTp, PADW)
            transpose_blocks(iv[:, :, WIN_PAD:WIN_PAD + CW], uh[s], on_dve)

        def t_back(s, on_dve=False):
            transpose_blocks(cview(uh[s])[:, :, :], poolT, on_dve)

        # ------------------------------------------------ sub-grid helpers
        def pe_sub(dst_seg_ap, src_full, scale=1.0, bias=0.0):
            """dst (sub scan-layout data view) <- src_full[::2,::2]*scale+bias."""
            sv = cview(src_full)
            ps = pmid.tile([128, NWS], f32, name="subps", tag="pmid")
            for cs in range(NCS):
                psc = ps[:, cs * WS:(cs + 1) * WS]
                nc.tensor.matmul(psc, cbf["selA"][:],
                                 sv[:, 2 * cs, 0:CW:2], start=True, stop=False)
                nc.tensor.matmul(psc, cbf["selB"][:],
                                 sv[:, 2 * cs + 1, 0:CW:2], start=False,
                                 stop=True)
            if scale == 1.0 and bias == 0.0:
                nc.scalar.copy(dst_seg_ap, cview(ps, WS)[:, :, :])
            else:
                nc.scalar.activation(dst_seg_ap, cview(ps, WS)[:, :, :],
                                     Act.Copy, bias=bias, scale=scale)

        def pe_sub_T(dst_seg_ap, scale, bias):
            """subsample from poolT (transposed pooled image) + re-transpose:
            avoids the full-res back-transpose of the dark2 pool."""
            pv = cview(poolT)
            ps = pmid.tile([128, NWS], f32, name="subTps", tag="pmid")
            for cs in range(NCS):
                psc = ps[:, cs * WS:(cs + 1) * WS]
                nc.tensor.matmul(psc, cbf["selA"][:],
                                 pv[:, 2 * cs, 0:CW:2], start=True, stop=False)
                nc.tensor.matmul(psc, cbf["selB"][:],
                                 pv[:, 2 * cs + 1, 0:CW:2], start=False,
                                 stop=True)
            tT = dout.tile([128, NWS], bf16, name="tT", tag="dout")
            nc.scalar.copy(tT[:], ps[:])
            ps2 = pmid.tile([128, NWS], bf16, name="subT2", tag="pmid")
            tv = cview(tT, WS)
            for ch in range(NCS):
                for cw in range(NCS):
                    nc.tensor.transpose(
                        ps2[:, ch * WS + cw * 128: ch * WS + (cw + 1) * 128],
                        tv[:, cw, ch * 128:(ch + 1) * 128], cbf["ident"][:])
            nc.scalar.activation(dst_seg_ap, cview(ps2, WS)[:, :, :],
                                 Act.Copy, bias=bias, scale=scale)

        def hbox_s(hb_t, src_t):
            cum = cump.tile([128, SCN_W], f32, name="cum", tag="cum")
            for c in range(NCS):
                V.tensor_tensor_scan(cum[:, c * SEG:(c + 1) * SEG],
                                     src_t[:, c * SEG:(c + 1) * SEG],
                                     fbcast(c_ones128[:, 0:1], SEG), 0.0,
                                     Alu.add, Alu.bypass)
            V.tensor_tensor(cview(hb_t, WS)[:, :, :],
                            segview(cum, LEAD + RS),
                            segview(cum, LEAD - RS - 1), Alu.subtract)

        def vbox_s(dst, src):
            sv = cview(src, WS)
            ps = pmid.tile([128, NWS], f32, name="vps", tag="pmid")
            r0 = ps[:, 0:WS]
            r1 = ps[:, WS:NWS]
            nc.tensor.matmul(r0, cbf["bm0s"][:], sv[:, 0, :], start=True,
                             stop=False)
            nc.tensor.matmul(r0, cbf["bds"][:], sv[:, 1, :], start=False,
                             stop=True)
            nc.tensor.matmul(r1, cbf["bm1s"][:], sv[:, 1, :], start=True,
                             stop=False)
            nc.tensor.matmul(r1, cbf["bus"][:], sv[:, 0, :], start=False,
                             stop=True)
            nc.scalar.copy(dst[:], ps[:])
            db = dst[:]
            edges = bass.AP(tensor=db.tensor, offset=db.offset,
                            ap=[list(db.ap[0]), [WS, NCS],
                                [WS - RS, 2], [1, RS]])
            fb = c_fixs[:]
            fv = bass.AP(tensor=fb.tensor, offset=fb.offset,
                         ap=[list(fb.ap[0]), [2 * RS, NCS], [RS, 2], [1, RS]])
            V.tensor_tensor(edges, edges, fv, Alu.mult)

        def upsample(dst_full_bf16, src_sub):
            """bilinear 2x upsample [128, 2x256] f32 -> [128, 4x512] bf16."""
            wide = mrot.tile([128, NCS * CW], bf16, name="wide", tag="wide",
                             bufs=2)
            wv = cview(wide)
            sv = cview(src_sub, WS)
            # W-upsample at sub rows
            wide_e = bass.AP(tensor=wv.tensor, offset=wv.offset,
                             ap=[list(wv.ap[0]), [CW, NCS], [2, WS]])
            V.tensor_copy(wide_e, sv[:, :, :])
            wide_o = bass.AP(tensor=wv.tensor, offset=wv.offset + 1,
                             ap=[list(wv.ap[0]), [CW, NCS], [2, WS - 1]])
            V.tensor_tensor(wide_o, sv[:, :, 0:WS - 1], sv[:, :, 1:WS], Alu.add)
            V.tensor_scalar(wide_o, wide_o, 0.5, 0.0, Alu.mult, Alu.add)
            lastc = bass.AP(tensor=wv.tensor, offset=wv.offset + CW - 1,
                            ap=[list(wv.ap[0]), [CW, NCS], [1, 1]])
            V.tensor_copy(lastc, sv[:, :, WS - 1:WS])
            # H-upsample via PE
            ps = pbig.tile([128, NW], f32, name="ups", tag="ptp")
            for c, mats in enumerate((
                    (("u00", 0),), (("u10", 0), ("u11", 1)),
                    (("u21", 1),), (("u31", 1),))):
                psc = ps[:, c * CW:(c + 1) * CW]
                for i, (nm, sc) in enumerate(mats):
                    nc.tensor.matmul(psc, cbf[nm][:], wv[:, sc, :],
                                     start=(i == 0), stop=(i == len(mats) - 1))
            nc.scalar.copy(dst_full_bf16[:], ps[:])

        # ---------------------------------------------- per-sample frontend
        ST = [dict(), dict()]

        def f_load(s):
            for chn in range(3):
                src = x_ext[s, chn].rearrange("(c p) w -> p c w", p=128)
                nc.gpsimd.dma_start(out=cview(x16[s][chn])[:, :, :], in_=src)

        def f_guid(s):
            gt = t_guid[s]
            tg = dout.tile([128, NW], bf16, name=f"gt{s}", tag="dout")
            tb = dout.tile([128, NW], bf16, name=f"bt{s}", tag="dout")
            nc.scalar.activation(gt[:], x16[s][0][:], Act.Copy,
                                 bias=0.5, scale=0.14945)
            nc.scalar.activation(tg[:], x16[s][1][:], Act.Copy,
                                 bias=0.0, scale=0.2935)
            nc.scalar.activation(tb[:], x16[s][2][:], Act.Copy,
                                 bias=0.0, scale=0.057)
            V.tensor_tensor(gt[:], gt[:], tg[:], Alu.add)
            V.tensor_tensor(gt[:], gt[:], tb[:], Alu.add)

        def f_chanmin_hpool(s, second):
            memset_pads(mxp, G)
            if not second:
                a0, a1, a2 = x16[s]
                V.tensor_tensor(interior(mxp), cview(a0)[:, :, :],
                                cview(a1)[:, :, :], Alu.min)
                V.tensor_tensor(interior(mxp), interior(mxp),
                                cview(a2)[:, :, :], Alu.min)
            else:
                chsc = ST[s]["chsc"]
                ytmp = junk
                nc.scalar.activation(interior(mxp), x16[s][0][:], Act.Identity,
                                     bias=chsc[:, 3:4], scale=chsc[:, 3:4])
                nc.scalar.activation(ytmp, x16[s][1][:], Act.Identity,
                                     bias=chsc[:, 4:5], scale=chsc[:, 4:5])
                V.tensor_tensor(interior(mxp), interior(mxp), junk_c, Alu.min)
                nc.scalar.activation(ytmp, x16[s][2][:], Act.Identity,
                                     bias=chsc[:, 5:6], scale=chsc[:, 5:6])
                V.tensor_tensor(interior(mxp), interior(mxp), junk_c, Alu.min)
            hpool(uh[s], mxp, w1)

        def f_hpoolT(s):
            hpool(poolT, uhTp, w1)

        def dark_phase(second):
            f_chanmin_hpool(0, second)
            t_fwd(0)
            f_chanmin_hpool(1, second)
            if not second:
                f_guid(0)
            f_hpoolT(0)
            t_back(0)
            t_fwd(1)
            if not second:
                f_guid(1)
            f_hpoolT(1)
            t_back(1)

        # ------------------------------------------------------- secant/topk
        def f_secant_init(s):
            st = ST[s]
            st["acc8"] = tiny.tile([128, 8], f32, name=f"acc8{s}", tag=f"acc8{s}")
            V.memset(st["acc8"][:], 0.0)
            st["thr"] = tiny.tile([128, 1], f32, name=f"thr{s}", tag=f"thr{s}")
            st["scal"] = tiny.tile([1, 16], f32, name=f"scal{s}", tag=f"scal{s}")
            V.memset(st["scal"][:], 0.0)
            V.memset(st["scal"][:, 0:1], T0)
            V.memset(st["scal"][:, 2:3], T1)

        def count_into(s, col, sub=False):
            st = ST[s]
            u, acc8, thr = uh[s], st["acc8"], st["thr"]
            uv = cview(u)
            if sub:
                V.tensor_scalar(junk_c[:, 0:2, 0:256],
                                uv[:, 0:NCHUNK:2, 0:CW:2], thr[:], 0.0,
                                Alu.is_gt, Alu.add,
                                accum_out=acc8[:, col:col + 1])
            else:
                V.tensor_scalar(junk, u[:, 0:NW], thr[:], 0.0,
                                Alu.is_gt, Alu.add,
                                accum_out=acc8[:, col:col + 1])
            fps = psml.tile([1, 1], f32, name=f"fold{s}", tag=f"fold{s}")
            nc.tensor.matmul(fps[:], c_ones128[:], acc8[:, col:col + 1],
                             start=True, stop=True)
            return fps

        def bcast_thr(s, src_col):
            st = ST[s]
            bp = psml.tile([128, 1], f32, name=f"thrps{s}", tag=f"fold{s}")
            nc.tensor.matmul(bp[:], c_ones1x[:], src_col, start=True, stop=True)
            nc.scalar.copy(st["thr"][:], bp[:])

        def f_count0(s, which):
            scal = ST[s]["scal"]
            col = 0 if which == 0 else 2
            bcast_thr(s, scal[0:1, col:col + 1])
            f = count_into(s, 0, sub=True)
            nc.scalar.copy(scal[:, col + 1:col + 2], f[:])

        def f_secant_round(s, rnd):
            scal = ST[s]["scal"]
            full = rnd >= SECANT_ROUNDS - 2
            if rnd == SECANT_ROUNDS - 2:
                V.tensor_scalar(scal[:, 1:2], scal[:, 1:2], 4.0, 0.0,
                                Alu.mult, Alu.add)
                V.tensor_scalar(scal[:, 3:4], scal[:, 3:4], 4.0, 0.0,
                                Alu.mult, Alu.add)
            V.tensor_tensor(scal[:, 4:5], scal[:, 3:4], scal[:, 1:2], Alu.subtract)
            V.tensor_scalar(scal[:, 8:9], scal[:, 4:5], -1.0, 0.0, Alu.mult, Alu.add)
            V.tensor_tensor(scal[:, 4:5], scal[:, 4:5], scal[:, 8:9], Alu.max)
            V.tensor_scalar(scal[:, 4:5], scal[:, 4:5], 1.0, 0.0, Alu.max, Alu.add)
            V.tensor_tensor(scal[:, 5:6], scal[:, 2:3], scal[:, 0:1], Alu.subtract)
            V.tensor_scalar(scal[:, 8:9], scal[:, 5:6], -1.0, 0.0, Alu.mult, Alu.add)
            V.tensor_tensor(scal[:, 5:6], scal[:, 5:6], scal[:, 8:9], Alu.max)
            V.reciprocal(scal[:, 8:9], scal[:, 4:5])
            V.tensor_tensor(scal[:, 5:6], scal[:, 5:6], scal[:, 8:9], Alu.mult)
            V.tensor_scalar(scal[:, 6:7], scal[:, 3:4], 1.0,
                            -float(TOPN) if full else -TOPN / 4.0,
                            Alu.mult, Alu.add)
            V.tensor_tensor(scal[:, 6:7], scal[:, 6:7], scal[:, 5:6], Alu.mult)
            V.tensor_copy(scal[:, 0:1], scal[:, 2:3])
            V.tensor_copy(scal[:, 1:2], scal[:, 3:4])
            V.tensor_tensor(scal[:, 2:3], scal[:, 2:3], scal[:, 6:7], Alu.add)
            bcast_thr(s, scal[0:1, 2:3])
            f = count_into(s, 0, sub=not full)
            nc.scalar.copy(scal[:, 3:4], f[:])

        def f_msums(s):
            st = ST[s]
            u, acc8, thr = uh[s], st["acc8"], st["thr"]
            V.tensor_scalar(junk, u[:, 0:NW], thr[:], 0.0,
                            Alu.is_gt, Alu.bypass)
            mbufs = (poolT[:], uhTp[:, 0:NW], mxp[:, 0:NW])
            for chn, xt in enumerate(x16[s]):
                mb = mbufs[chn % 3]
                V.tensor_tensor(mb, junk, xt[:], Alu.mult)
                nc.scalar.activation(mb, mb, Act.Copy,
                                     accum_out=acc8[:, 1 + chn:2 + chn])

        def f_bandprep(s):
            st = ST[s]
            scal = st["scal"]
            V.tensor_scalar(scal[:, 7:8], scal[:, 2:3], 1.0, -BAND,
                            Alu.mult, Alu.add)
            bcast_thr(s, scal[0:1, 7:8])

        def f_bandsums(s):
            st = ST[s]
            u, acc8, thr = uh[s], st["acc8"], st["thr"]
            V.tensor_scalar(junk, u[:, 0:NW], thr[:], 0.0,
                            Alu.is_gt, Alu.bypass)
            nc.scalar.activation(poolT[:], junk, Act.Copy,
                                 accum_out=acc8[:, 4:5])
            mbufs = (poolT[:], uhTp[:, 0:NW], mxp[:, 0:NW])
            for chn, xt in enumerate(x16[s]):
                mb = mbufs[chn % 3]
                V.tensor_tensor(mb, junk, xt[:], Alu.mult)
                nc.scalar.activation(mb, mb, Act.Copy,
                                     accum_out=acc8[:, 5 + chn:6 + chn])

        def f_afold(s):
            st = ST[s]
            tps = psml.tile([1, 8], f32, name=f"totps{s}", tag=f"fold{s}")
            nc.tensor.matmul(tps[:], c_ones128[:], st["acc8"][:],
                             start=True, stop=True)
            tot = tiny.tile([1, 8], f32, name=f"tot{s}", tag=f"tot{s}")
            nc.scalar.copy(tot[:], tps[:])
            st["tot"] = tot

        def f_amath(s):
            st = ST[s]
            tot = st["tot"]
            am = tiny.tile([1, 12], f32, name=f"am{s}", tag=f"am{s}")
            V.tensor_tensor(am[:, 0:3], tot[:, 5:8], tot[:, 1:4], Alu.subtract)
            V.tensor_tensor(am[:, 11:12], tot[:, 4:5], tot[:, 0:1], Alu.subtract)
            V.tensor_scalar(am[:, 11:12], am[:, 11:12], 1.0, 0.0, Alu.max, Alu.add)
            V.reciprocal(am[:, 10:11], am[:, 11:12])
            V.tensor_tensor(am[:, 0:3], am[:, 0:3], fbcast(am[:, 10:11], 3), Alu.mult)
            V.tensor_scalar(am[:, 9:10], tot[:, 0:1], -1.0, float(TOPN),
                            Alu.mult, Alu.add)
            V.tensor_tensor(am[:, 0:3], am[:, 0:3], fbcast(am[:, 9:10], 3), Alu.mult)
            V.tensor_tensor(am[:, 0:3], am[:, 0:3], tot[:, 1:4], Alu.add)
            V.tensor_scalar(am[:, 0:3], am[:, 0:3], 1.0 / TOPN, 0.0, Alu.mult, Alu.add)
            V.tensor_scalar(am[:, 3:6], am[:, 0:3], 1.0, 1.0, Alu.mult, Alu.add)
            V.reciprocal(am[:, 3:6], am[:, 3:6])
            V.tensor_scalar(am[:, 0:3], am[:, 0:3], 0.5, 0.5, Alu.mult, Alu.add)
            V.tensor_scalar(am[:, 6:9], am[:, 0:3], -1.0, 0.5, Alu.mult, Alu.add)
            st["am"] = am

        def f_chsc(s):
            st = ST[s]
            st["chsc"] = tiny.tile([128, 9], f32, name=f"chsc{s}",
                                   tag=f"chsc{s}")
            bp = psml.tile([128, 9], f32, name=f"chps{s}", tag=f"fold{s}")
            nc.tensor.matmul(bp[:], c_ones1x[:], st["am"][0:1, 0:9],
                             start=True, stop=True)
            nc.scalar.copy(st["chsc"][:], bp[:])

        # ------------------------------------------- guidance-only box prep
        def prep_ops(s):
            yield lambda: pe_sub(sview(t_is[s]), t_guid[s])
            yield lambda: nc.scalar.activation(sview(t_ii[s]), sview(t_is[s]),
                                               Act.Square)
            hbI = [None]
            hbII = [None]
            mII = [None]

            def scanI():
                hbI[0] = rot.tile([128, NWS], f32r, name="hbI", tag="hbx")
                hbox_s(hbI[0], t_is[s])
            yield scanI
            yield lambda: vbox_s(mean_Is[s], hbI[0])

            def scanII():
                hbII[0] = rot.tile([128, NWS], f32r, name="hbII", tag="hbx")
                hbox_s(hbII[0], t_ii[s])
            yield scanII

            def vboxII():
                mII[0] = mrot.tile([128, NWS], f32, name="mII", tag="mpx")
                vbox_s(mII[0], hbII[0])
            yield vboxII

            def varrec():
                sq = sab.tile([128, NWS], f32, name="sq", tag="sab")
                nc.scalar.activation(sq[:], mean_Is[s][:], Act.Square)
                V.scalar_tensor_tensor(sq[:], mII[0][:], EPS, sq[:],
                                       Alu.add, Alu.subtract)
                V.reciprocal_approx_fast(out=rec_s[s][:], in_=sq[:])
            yield varrec

        # ---------------------------------------------------------- backend
        BK = [dict(), dict()]

        def backend_head(s):
            pe_sub(sview(t_ps[s]), uh[s], scale=-OMEGA, bias=1.0)
            V.tensor_tensor(sview(t_ip[s]), sview(t_is[s]), sview(t_ps[s]),
                            Alu.mult)
            hb_p = rot.tile([128, NWS], f32r, name="hb_p", tag="hbx")
            hbox_s(hb_p, t_ps[s])
            mean_p = mrot.tile([128, NWS], f32, name="mean_p", tag="mpx")
            vbox_s(mean_p, hb_p)
            hb_ip = rot.tile([128, NWS], f32r, name="hb_ip", tag="hbx")
            hbox_s(hb_ip, t_ip[s])
            mean_Ip = mrot.tile([128, NWS], f32, name="mean_Ip", tag="mpx")
            vbox_s(mean_Ip, hb_ip)
            BK[s]["mp"], BK[s]["mip"] = mean_p, mean_Ip

        def backend_mid(s):
            mean_p, mean_Ip = BK[s]["mp"], BK[s]["mip"]
            tmp = sab.tile([128, NWS], f32, name="tmp", tag="sab")
            V.tensor_tensor(tmp[:], mean_Is[s][:], mean_p[:], Alu.mult)
            cov = sab.tile([128, NWS], f32, name="cov", tag="sab")
            V.tensor_tensor(cov[:], mean_Ip[:], tmp[:], Alu.subtract)
            a_v = sview(t_ip[s])          # overwrite Ip (dead) with a
            V.tensor_tensor(a_v, cview(cov, WS)[:, :, :],
                            cview(rec_s[s], WS)[:, :, :], Alu.mult)
            t2 = sab.tile([128, NWS], f32, name="t2", tag="sab")
            V.tensor_tensor(cview(t2, WS)[:, :, :], a_v,
                            cview(mean_Is[s], WS)[:, :, :], Alu.mult)
            b_v = sview(t_ps[s])          # overwrite p (dead) with b
            V.tensor_tensor(b_v, cview(mean_p, WS)[:, :, :],
                            cview(t2, WS)[:, :, :], Alu.subtract)

            hba = rot.tile([128, NWS], f32r, name="hba", tag="hbx")
            hbox_s(hba, t_ip[s])
            mean_a = mrot.tile([128, NWS], f32, name="mean_a", tag="mpx")
            vbox_s(mean_a, hba)
            hbb = rot.tile([128, NWS], f32r, name="hbb", tag="hbx")
            hbox_s(hbb, t_ps[s])
            mean_b = mrot.tile([128, NWS], f32, name="mean_b", tag="mpx")
            vbox_s(mean_b, hbb)
            ma_f = mfull.tile([128, NW], bf16, name="ma_f", tag="mf")
            upsample(ma_f, mean_a)
            mb_f = mfull.tile([128, NW], bf16, name="mb_f", tag="mf")
            upsample(mb_f, mean_b)
            BK[s]["ma"], BK[s]["mb"] = ma_f, mb_f

        def backend_tail(s):
            chsc = ST[s]["chsc"]
            ma_f, mb_f = BK[s]["ma"], BK[s]["mb"]
            HW_ = NW // 2
            T16 = dout.tile([128, NW], bf16, name="T16", tag="dout")
            T_t = abt.tile([128, NW], f32, name="T_t", tag="abt")
            rT = abt.tile([128, NW], f32, name="rT", tag="abt")
            for h in (0, 1):
                sl = slice(h * HW_, (h + 1) * HW_)
                V.tensor_tensor(T16[:, sl], ma_f[:, sl], t_guid[s][:, sl],
                                Alu.mult)
                V.tensor_tensor(T16[:, sl], T16[:, sl], mb_f[:, sl], Alu.add)
                nc.scalar.copy(T_t[:, sl], T16[:, sl])
                V.reciprocal_approx_fast(out=rT[:, sl], in_=T_t[:, sl])
                if s == 0:
                    nc.scalar.copy(poolT[:, sl], rT[:, sl])
            rmul = poolT if s == 0 else rT

            for chn in range(3):
                d_t = dout.tile([128, NW], bf16, name=f"d{chn}", tag="dout")
                yv = y_ext[s, chn].rearrange("(c p) w -> p c w", p=128)
                for h in (0, 1):
                    sl = slice(h * HW_, (h + 1) * HW_)
                    nc.scalar.activation(d_t[:, sl], x16[s][chn][:, sl],
                                         Act.Identity,
                                         bias=chsc[:, 6 + chn:7 + chn],
                                         scale=0.5)
                    V.tensor_tensor(d_t[:, sl], d_t[:, sl], rmul[:, sl],
                                    Alu.mult)
                    V.tensor_scalar(d_t[:, sl], d_t[:, sl],
                                    chsc[:, chn:chn + 1], 0.0,
                                    Alu.add, Alu.add)
                    nc.gpsimd.dma_start(out=yv[:, 2 * h:2 * h + 2, :],
                                        in_=cview(d_t)[:, 2 * h:2 * h + 2, :])

        # ================================================== emission order
        f_load(0)
        f_load(1)
        load_consts()
        dark_phase(second=False)

        f_secant_init(0)
        f_secant_init(1)
        preps = list(prep_ops(0)) + list(prep_ops(1))
        pi = 0

        def drain_prep(n=1):
            nonlocal pi
            for _ in range(n):
                if pi < len(preps):
                    preps[pi]()
                    pi += 1

        for which in (0, 1):
            f_count0(0, which)
            drain_prep()
            f_count0(1, which)
            drain_prep()
        for rnd in range(SECANT_ROUNDS):
            f_secant_round(0, rnd)
            drain_prep()
            f_secant_round(1, rnd)
            drain_prep()
        f_msums(0)
        f_msums(1)
        f_bandprep(0)
        f_bandprep(1)
        drain_prep(2)
        f_bandsums(0)
        f_bandsums(1)
        drain_prep(len(preps))
        f_afold(0)
        f_afold(1)
        f_amath(0)
        f_amath(1)
        f_chsc(0)
        f_chsc(1)

        # dark2 phase with backend_head(0) interleaved after sample 0's pool
        f_chanmin_hpool(0, True)
        t_fwd(0)
        f_chanmin_hpool(1, True)
        f_hpoolT(0)
        t_back(0)
        backend_head(0)
        t_fwd(1)
        f_hpoolT(1)
        t_back(1)
        backend_mid(0)
        backend_head(1)
        backend_tail(0)
        backend_mid(1)
        backend_tail(1)

    nc.compile()
    return nc


def _get_program():
    if "nc" not in _CACHE:
        _CACHE["nc"] = _build()
    return _CACHE["nc"]


def kernel(x: np.ndarray) -> np.ndarray:
    from concourse.bass_utils import run_bass_kernel_spmd
    x = np.ascontiguousarray(np.asarray(x, dtype=np.float32))
    assert x.shape == (16, 3, H, W), x.shape
    nc = _get_program()
    consts = _host_consts()
    in_maps = [{"x": x[2 * i:2 * i + 2], **consts} for i in range(8)]
    res = run_bass_kernel_spmd(nc, in_maps, list(range(8)))
    out = np.concatenate([res.results[i]["y"] for i in range(8)], axis=0)
    return out.astype(np.float32)


# revision 61
# speedup vs baseline: 1.0347x; 1.0091x over previous
"""Dark-Channel-Prior dehazing (DCPGenerator) Trainium2 Bass kernel, v9.

v8 -> v9: the guided filter runs as a fast-guided-filter at 2x subsample
(256x256): all six box filters (I, II, p, Ip, a, b), the cov/var/a/b
math, and the vbox matmuls operate on 1/4 the pixels with radius-20
bands; mean_a/mean_b are bilinearly upsampled (PE matmuls for rows, DVE
for columns) and T = mean_a*I + mean_b is applied at full resolution.
Subsampling of guid / pooled-dark runs on the PE with selection
matrices.  Dark channel, top-k secant, and A estimation stay full-res.
"""
import numpy as np
from contextlib import ExitStack

H = 512
W = 512
NCHUNK = 4
CW = 512
NW = NCHUNK * CW            # 2048
PADW = 526                  # 7 | 512 | 7
WIN_PAD = 7
RADIUS = 40
# sub-grid (fast guided filter, s=2)
HS = 256
WS = 256
NCS = 2
RS = 20
LEAD = 24                   # leading zeros in sub scan layout (>=RS+1)
SEG = 300                   # WS + 44-zero gap (>= 2*RS+1)
SCN_W = LEAD + NCS * SEG    # 624
NWS = NCS * WS              # 512
EPS = 1e-3
OMEGA = 0.95
TOPN = int(0.01 * H * W)    # 2621
T0 = 0.0055
T1 = 0.0085
BAND = 2e-4
SECANT_ROUNDS = 5

_CACHE = {}


def _host_consts():
    # full-res H-direction box weights are no longer needed; sub-grid ones:
    i = np.arange(HS)
    n1s = np.minimum(i + RS, HS - 1) - np.maximum(i - RS, 0) + 1
    inv_ns = (1.0 / n1s).astype(np.float32)
    k = np.arange(128)[:, None]
    p = np.arange(128)[None, :]
    bands = (np.abs(k - p) <= RS).astype(np.float32)
    bus = (k >= p + 128 - RS).astype(np.float32) / 41.0 / 41.0
    bds = (k <= p - (128 - RS)).astype(np.float32) / 41.0 / 41.0
    bm0s = bands * inv_ns[0:128][None, :] / 41.0
    bm1s = bands * inv_ns[128:256][None, :] / 41.0
    fix40 = np.concatenate([41.0 * inv_ns[0:RS], 41.0 * inv_ns[WS - RS:]])
    fixs = np.tile(fix40[None, :], (128, NCS)).copy()        # [128, 80]
    ident = np.eye(128, dtype=np.float32)
    # row-subsample selection: out q <- full partition 2q (two half matrices)
    selA = np.zeros((128, 128), np.float32)
    selB = np.zeros((128, 128), np.float32)
    for q in range(64):
        selA[2 * q, q] = 1.0
    for q in range(64, 128):
        selB[2 * (q - 64), q] = 1.0
    # row-upsample (bilinear, sub sample i at full row 2i)
    U = {}
    for c in range(NCHUNK):
        for q in range(128):
            r = 128 * c + q
            if r % 2 == 0:
                pairs = [(r // 2, 1.0)]
            else:
                i0 = (r - 1) // 2
                i1 = min(i0 + 1, HS - 1)
                pairs = [(i0, 0.5), (i1, 0.5)] if i1 != i0 else [(i0, 1.0)]
            for i_, wgt in pairs:
                sc, pp_ = divmod(i_, 128)
                U.setdefault((c, sc), np.zeros((128, 128), np.float32))[
                    pp_, q] += wgt
    return {"bm0s": bm0s, "bm1s": bm1s, "bus": bus, "bds": bds,
            "fixs": fixs, "ident": ident, "selA": selA, "selB": selB,
            "u00": U[(0, 0)], "u10": U[(1, 0)], "u11": U[(1, 1)],
            "u21": U[(2, 1)], "u31": U[(3, 1)]}


def _build():
    import concourse.bacc as bacc
    import concourse.tile as tile
    import concourse.bass as bass
    from concourse import mybir

    f32 = mybir.dt.float32
    f32r = mybir.dt.float32r
    bf16 = mybir.dt.bfloat16
    Alu = mybir.AluOpType
    Act = mybir.ActivationFunctionType

    nc = bacc.Bacc("TRN2", target_bir_lowering=False, debug=False, num_devices=8)
    V = nc.vector
    G = nc.gpsimd

    x_ext = nc.dram_tensor("x", [2, 3, H, W], f32, kind="ExternalInput").ap()
    c128_names = ("bm0s", "bm1s", "bus", "bds", "ident", "selA", "selB",
                  "u00", "u10", "u11", "u21", "u31")
    c128_exts = {nm: nc.dram_tensor(nm, [128, 128], f32, kind="ExternalInput").ap()
                 for nm in c128_names}
    fixs_ext = nc.dram_tensor("fixs", [128, NCS * 2 * RS], f32,
                              kind="ExternalInput").ap()
    y_ext = nc.dram_tensor("y", [2, 3, H, W], f32, kind="ExternalOutput").ap()

    def cview(t, width=CW):
        return t.rearrange("p (c w) -> p c w", w=width)

    def fbcast(ap_col, n):
        return bass.AP(tensor=ap_col.tensor, offset=ap_col.offset,
                       ap=[list(p) for p in ap_col.ap[:-1]] + [[0, n]])

    def segview(t, off, c0=0, nch=NCS):
        """[128, nch, WS] view into a [128, SCN_W] sub tile."""
        base = t[:]
        return bass.AP(tensor=base.tensor, offset=base.offset + off + c0 * SEG,
                       ap=[list(base.ap[0]), [SEG, nch], [1, WS]])

    def sview(t):
        return segview(t, LEAD)

    with ExitStack() as ctx:
        tc = ctx.enter_context(tile.TileContext(nc))

        cpool = ctx.enter_context(tc.tile_pool(name="cpool", bufs=1))
        srcp = ctx.enter_context(tc.tile_pool(name="srcp", bufs=1))
        scn = ctx.enter_context(tc.tile_pool(name="scn", bufs=1))
        pp = ctx.enter_context(tc.tile_pool(name="pp", bufs=1))
        cump = ctx.enter_context(tc.tile_pool(name="cump", bufs=2))
        boxes = ctx.enter_context(tc.tile_pool(name="boxes", bufs=1))
        rot = ctx.enter_context(tc.tile_pool(name="rot", bufs=2))
        mrot = ctx.enter_context(tc.tile_pool(name="mrot", bufs=4))
        abt = ctx.enter_context(tc.tile_pool(name="abt", bufs=3))
        sab = ctx.enter_context(tc.tile_pool(name="sab", bufs=3))
        dout = ctx.enter_context(tc.tile_pool(name="dout", bufs=2))
        mfull = ctx.enter_context(tc.tile_pool(name="mfull", bufs=2))
        tiny = ctx.enter_context(tc.tile_pool(name="tiny", bufs=1))
        pbig = ctx.enter_context(tc.tile_pool(name="pbig", bufs=1, space="PSUM"))
        pmid = ctx.enter_context(tc.tile_pool(name="pmid", bufs=2, space="PSUM"))
        psml = ctx.enter_context(tc.tile_pool(name="psml", bufs=1, space="PSUM"))

        # ------------------------------------- constants (loaded after x DMAs)
        cbf = {}
        stage = cpool.tile([128, 128], f32, name="s_band")
        for nm in ("bm0s", "bm1s", "bus", "bds"):
            cbf[nm] = cpool.tile([128, 128], f32r, name=f"c_{nm}")
        for nm in ("ident", "selA", "selB", "u00", "u10", "u11", "u21", "u31"):
            cbf[nm] = cpool.tile([128, 128], bf16, name=f"c_{nm}")
        c_fixs = cpool.tile([128, NCS * 2 * RS], f32, name="c_fixs")
        c_ones128 = cpool.tile([128, 1], f32, name="c_ones128")
        c_ones1x = cpool.tile([1, 128], f32, name="c_ones1x")

        def load_consts():
            for nm in ("bm0s", "bm1s", "bus", "bds", "ident", "selA", "selB",
                       "u00", "u10", "u11", "u21", "u31"):
                nc.sync.dma_start(out=stage[:], in_=c128_exts[nm][:])
                nc.scalar.copy(cbf[nm][:], stage[:])
            nc.sync.dma_start(out=c_fixs[:], in_=fixs_ext[:])
            V.memset(c_ones128[:], 1.0)
            V.memset(c_ones1x[:], 1.0)

        # --------------------------------------------------- persistent tiles
        x16 = [[srcp.tile([128, NW], bf16, name=f"x16_{s}_{c}")
                for c in range(3)] for s in range(2)]
        t_guid = [srcp.tile([128, NW], bf16, name=f"guid{s}") for s in range(2)]
        # sub-grid scan-layout sources (f32): I, p, Ip, II, a, b per sample
        t_is = [scn.tile([128, SCN_W], f32, name=f"is{s}") for s in range(2)]
        t_ps = [scn.tile([128, SCN_W], f32, name=f"ps{s}") for s in range(2)]
        t_ip = [scn.tile([128, SCN_W], f32, name=f"ip{s}") for s in range(2)]
        t_ii = [scn.tile([128, SCN_W], f32, name=f"ii{s}") for s in range(2)]
        mxp = pp.tile([128, NCHUNK * PADW], bf16, name="mxp")
        w1 = pp.tile([128, NCHUNK * PADW], bf16, name="w1")
        uhTp = pp.tile([128, NCHUNK * PADW], bf16, name="uhTp")
        poolT = pp.tile([128, NW], bf16, name="poolT")
        uh = [pp.tile([128, NW], bf16, name=f"uh{s}") for s in range(2)]
        mean_Is = [boxes.tile([128, NWS], f32, name=f"meanIs{s}")
                   for s in range(2)]
        rec_s = [boxes.tile([128, NWS], f32, name=f"recs{s}") for s in range(2)]

        junkt = pp.tile([128, NW], bf16, name="junkt")
        junk = junkt[:, 0:NW]
        junk_c = junk.rearrange("p (c w) -> p c w", w=CW)

        # zero the sub scan-layout gaps once
        for t in (t_is[0], t_is[1], t_ps[0], t_ps[1], t_ip[0], t_ip[1],
                  t_ii[0], t_ii[1]):
            V.memset(t[:, 0:LEAD], 0.0)
            for c in range(NCS):
                V.memset(t[:, LEAD + c * SEG + WS: LEAD + (c + 1) * SEG], 0.0)

        # ---------------------------------------------------------- helpers
        def interior(t):
            return cview(t, PADW)[:, :, WIN_PAD:WIN_PAD + CW]

        def memset_pads(t, eng):
            v = cview(t, PADW)
            for c in range(NCHUNK):
                eng.memset(v[:, c, 0:WIN_PAD], 1.0)
                eng.memset(v[:, c, PADW - WIN_PAD:PADW], 1.0)

        def hpool(dst, padded, scratch):
            a = cview(padded, PADW)
            b = cview(scratch, PADW)
            d = cview(dst)
            V.tensor_tensor(b[:, :, 0:525], a[:, :, 0:525], a[:, :, 1:526], Alu.min)
            V.tensor_tensor(a[:, :, 0:523], b[:, :, 0:523], b[:, :, 2:525], Alu.min)
            V.tensor_tensor(b[:, :, 0:519], a[:, :, 0:519], a[:, :, 4:523], Alu.min)
            V.tensor_tensor(d[:, 0:NCHUNK, :], b[:, :, 0:512], b[:, :, 7:519],
                            Alu.min)

        def transpose_blocks(dst_ap, src_flat, on_dve=False):
            sv = cview(src_flat)
            pt = pbig.tile([128, NW], bf16, name="pt", tag="ptp")
            for co in range(NCHUNK):
                for ci in range(NCHUNK):
                    nc.tensor.transpose(
                        pt[:, co * CW + ci * 128: co * CW + (ci + 1) * 128],
                        sv[:, ci, co * 128:(co + 1) * 128], cbf["ident"][:])
            if on_dve:
                V.tensor_copy(dst_ap, cview(pt)[:, :, :])
            else:
                nc.scalar.copy(dst_ap, cview(pt)[:, :, :])

        def t_fwd(s, on_dve=False):
            memset_pads(uhTp, G)
            iv = cview(uhTp, PADW)
            transpose_blocks(iv[:, :, WIN_PAD:WIN_PAD + CW], uh[s], on_dve)

        def t_back(s, on_dve=False):
            transpose_blocks(cview(uh[s])[:, :, :], poolT, on_dve)

        # ------------------------------------------------ sub-grid helpers
        def pe_sub(dst_seg_ap, src_full, scale=1.0, bias=0.0):
            """dst (sub scan-layout data view) <- src_full[::2,::2]*scale+bias."""
            sv = cview(src_full)
            ps = pmid.tile([128, NWS], f32, name="subps", tag="pmid")
            for cs in range(NCS):
                psc = ps[:, cs * WS:(cs + 1) * WS]
                nc.tensor.matmul(psc, cbf["selA"][:],
                                 sv[:, 2 * cs, 0:CW:2], start=True, stop=False)
                nc.tensor.matmul(psc, cbf["selB"][:],
                                 sv[:, 2 * cs + 1, 0:CW:2], start=False,
                                 stop=True)
            if scale == 1.0 and bias == 0.0:
                nc.scalar.copy(dst_seg_ap, cview(ps, WS)[:, :, :])
            else:
                nc.scalar.activation(dst_seg_ap, cview(ps, WS)[:, :, :],
                                     Act.Copy, bias=bias, scale=scale)

        def pe_sub_T(dst_seg_ap, scale, bias):
            """subsample from poolT (transposed pooled image) + re-transpose:
            avoids the full-res back-transpose of the dark2 pool."""
            pv = cview(poolT)
            ps = pmid.tile([128, NWS], f32, name="subTps", tag="pmid")
            for cs in range(NCS):
                psc = ps[:, cs * WS:(cs + 1) * WS]
                nc.tensor.matmul(psc, cbf["selA"][:],
                                 pv[:, 2 * cs, 0:CW:2], start=True, stop=False)
                nc.tensor.matmul(psc, cbf["selB"][:],
                                 pv[:, 2 * cs + 1, 0:CW:2], start=False,
                                 stop=True)
            tT = dout.tile([128, NWS], bf16, name="tT", tag="dout")
            nc.scalar.copy(tT[:], ps[:])
            ps2 = pmid.tile([128, NWS], bf16, name="subT2", tag="pmid")
            tv = cview(tT, WS)
            for ch in range(NCS):
                for cw in range(NCS):
                    nc.tensor.transpose(
                        ps2[:, ch * WS + cw * 128: ch * WS + (cw + 1) * 128],
                        tv[:, cw, ch * 128:(ch + 1) * 128], cbf["ident"][:])
            nc.scalar.activation(dst_seg_ap, cview(ps2, WS)[:, :, :],
                                 Act.Copy, bias=bias, scale=scale)

        def hbox_s(hb_t, src_t):
            cum = cump.tile([128, SCN_W], f32, name="cum", tag="cum")
            for c in range(NCS):
                V.tensor_tensor_scan(cum[:, c * SEG:(c + 1) * SEG],
                                     src_t[:, c * SEG:(c + 1) * SEG],
                                     fbcast(c_ones128[:, 0:1], SEG), 0.0,
                                     Alu.add, Alu.bypass)
            V.tensor_tensor(cview(hb_t, WS)[:, :, :],
                            segview(cum, LEAD + RS),
                            segview(cum, LEAD - RS - 1), Alu.subtract)

        def vbox_s(dst, src):
            sv = cview(src, WS)
            ps = pmid.tile([128, NWS], f32, name="vps", tag="pmid")
            r0 = ps[:, 0:WS]
            r1 = ps[:, WS:NWS]
            nc.tensor.matmul(r0, cbf["bm0s"][:], sv[:, 0, :], start=True,
                             stop=False)
            nc.tensor.matmul(r0, cbf["bds"][:], sv[:, 1, :], start=False,
                             stop=True)
            nc.tensor.matmul(r1, cbf["bm1s"][:], sv[:, 1, :], start=True,
                             stop=False)
            nc.tensor.matmul(r1, cbf["bus"][:], sv[:, 0, :], start=False,
                             stop=True)
            nc.scalar.copy(dst[:], ps[:])
            db = dst[:]
            edges = bass.AP(tensor=db.tensor, offset=db.offset,
                            ap=[list(db.ap[0]), [WS, NCS],
                                [WS - RS, 2], [1, RS]])
            fb = c_fixs[:]
            fv = bass.AP(tensor=fb.tensor, offset=fb.offset,
                         ap=[list(fb.ap[0]), [2 * RS, NCS], [RS, 2], [1, RS]])
            V.tensor_tensor(edges, edges, fv, Alu.mult)

        def upsample(dst_full_bf16, src_sub):
            """bilinear 2x upsample [128, 2x256] f32 -> [128, 4x512] bf16."""
            wide = mrot.tile([128, NCS * CW], bf16, name="wide", tag="wide",
                             bufs=2)
            wv = cview(wide)
            sv = cview(src_sub, WS)
            # W-upsample at sub rows
            wide_e = bass.AP(tensor=wv.tensor, offset=wv.offset,
                             ap=[list(wv.ap[0]), [CW, NCS], [2, WS]])
            V.tensor_copy(wide_e, sv[:, :, :])
            wide_o = bass.AP(tensor=wv.tensor, offset=wv.offset + 1,
                             ap=[list(wv.ap[0]), [CW, NCS], [2, WS - 1]])
            V.tensor_tensor(wide_o, sv[:, :, 0:WS - 1], sv[:, :, 1:WS], Alu.add)
            V.tensor_scalar(wide_o, wide_o, 0.5, 0.0, Alu.mult, Alu.add)
            lastc = bass.AP(tensor=wv.tensor, offset=wv.offset + CW - 1,
                            ap=[list(wv.ap[0]), [CW, NCS], [1, 1]])
            V.tensor_copy(lastc, sv[:, :, WS - 1:WS])
            # H-upsample via PE
            ps = pbig.tile([128, NW], f32, name="ups", tag="ptp")
            for c, mats in enumerate((
                    (("u00", 0),), (("u10", 0), ("u11", 1)),
                    (("u21", 1),), (("u31", 1),))):
                psc = ps[:, c * CW:(c + 1) * CW]
                for i, (nm, sc) in enumerate(mats):
                    nc.tensor.matmul(psc, cbf[nm][:], wv[:, sc, :],
                                     start=(i == 0), stop=(i == len(mats) - 1))
            nc.scalar.copy(dst_full_bf16[:], ps[:])

        # ---------------------------------------------- per-sample frontend
        ST = [dict(), dict()]

        def f_load(s):
            for chn in range(3):
                src = x_ext[s, chn].rearrange("(c p) w -> p c w", p=128)
                nc.gpsimd.dma_start(out=cview(x16[s][chn])[:, :, :], in_=src)

        def f_guid(s):
            gt = t_guid[s]
            tg = dout.tile([128, NW], bf16, name=f"gt{s}", tag="dout")
            tb = dout.tile([128, NW], bf16, name=f"bt{s}", tag="dout")
            nc.scalar.activation(gt[:], x16[s][0][:], Act.Copy,
                                 bias=0.5, scale=0.14945)
            nc.scalar.activation(tg[:], x16[s][1][:], Act.Copy,
                                 bias=0.0, scale=0.2935)
            nc.scalar.activation(tb[:], x16[s][2][:], Act.Copy,
                                 bias=0.0, scale=0.057)
            V.tensor_tensor(gt[:], gt[:], tg[:], Alu.add)
            V.tensor_tensor(gt[:], gt[:], tb[:], Alu.add)

        def f_chanmin_hpool(s, second):
            memset_pads(mxp, G)
            if not second:
                a0, a1, a2 = x16[s]
                V.tensor_tensor(interior(mxp), cview(a0)[:, :, :],
                                cview(a1)[:, :, :], Alu.min)
                V.tensor_tensor(interior(mxp), interior(mxp),
                                cview(a2)[:, :, :], Alu.min)
            else:
                chsc = ST[s]["chsc"]
                ytmp = junk
                nc.scalar.activation(interior(mxp), x16[s][0][:], Act.Identity,
                                     bias=chsc[:, 3:4], scale=chsc[:, 3:4])
                nc.scalar.activation(ytmp, x16[s][1][:], Act.Identity,
                                     bias=chsc[:, 4:5], scale=chsc[:, 4:5])
                V.tensor_tensor(interior(mxp), interior(mxp), junk_c, Alu.min)
                nc.scalar.activation(ytmp, x16[s][2][:], Act.Identity,
                                     bias=chsc[:, 5:6], scale=chsc[:, 5:6])
                V.tensor_tensor(interior(mxp), interior(mxp), junk_c, Alu.min)
            hpool(uh[s], mxp, w1)

        def f_hpoolT(s):
            hpool(poolT, uhTp, w1)

        def dark_phase(second):
            f_chanmin_hpool(0, second)
            t_fwd(0)
            f_chanmin_hpool(1, second)
            if not second:
                f_guid(0)
            f_hpoolT(0)
            t_back(0)
            t_fwd(1)
            if not second:
                f_guid(1)
            f_hpoolT(1)
            t_back(1)

        # ------------------------------------------------------- secant/topk
        def f_secant_init(s):
            st = ST[s]
            st["acc8"] = tiny.tile([128, 8], f32, name=f"acc8{s}", tag=f"acc8{s}")
            V.memset(st["acc8"][:], 0.0)
            st["thr"] = tiny.tile([128, 1], f32, name=f"thr{s}", tag=f"thr{s}")
            st["scal"] = tiny.tile([1, 16], f32, name=f"scal{s}", tag=f"scal{s}")
            V.memset(st["scal"][:], 0.0)
            V.memset(st["scal"][:, 0:1], T0)
            V.memset(st["scal"][:, 2:3], T1)

        def count_into(s, col, sub=False):
            st = ST[s]
            u, acc8, thr = uh[s], st["acc8"], st["thr"]
            uv = cview(u)
            if sub:
                V.tensor_scalar(junk_c[:, 0:2, 0:256],
                                uv[:, 0:NCHUNK:2, 0:CW:2], thr[:], 0.0,
                                Alu.is_gt, Alu.add,
                                accum_out=acc8[:, col:col + 1])
            else:
                V.tensor_scalar(junk, u[:, 0:NW], thr[:], 0.0,
                                Alu.is_gt, Alu.add,
                                accum_out=acc8[:, col:col + 1])
            fps = psml.tile([1, 1], f32, name=f"fold{s}", tag=f"fold{s}")
            nc.tensor.matmul(fps[:], c_ones128[:], acc8[:, col:col + 1],
                             start=True, stop=True)
            return fps

        def bcast_thr(s, src_col):
            st = ST[s]
            bp = psml.tile([128, 1], f32, name=f"thrps{s}", tag=f"fold{s}")
            nc.tensor.matmul(bp[:], c_ones1x[:], src_col, start=True, stop=True)
            nc.scalar.copy(st["thr"][:], bp[:])

        def f_count0(s, which):
            scal = ST[s]["scal"]
            col = 0 if which == 0 else 2
            bcast_thr(s, scal[0:1, col:col + 1])
            f = count_into(s, 0, sub=True)
            nc.scalar.copy(scal[:, col + 1:col + 2], f[:])

        def f_secant_round(s, rnd):
            scal = ST[s]["scal"]
            full = rnd >= SECANT_ROUNDS - 2
            if rnd == SECANT_ROUNDS - 2:
                V.tensor_scalar(scal[:, 1:2], scal[:, 1:2], 4.0, 0.0,
                                Alu.mult, Alu.add)
                V.tensor_scalar(scal[:, 3:4], scal[:, 3:4], 4.0, 0.0,
                                Alu.mult, Alu.add)
            V.tensor_tensor(scal[:, 4:5], scal[:, 3:4], scal[:, 1:2], Alu.subtract)
            V.tensor_scalar(scal[:, 8:9], scal[:, 4:5], -1.0, 0.0, Alu.mult, Alu.add)
            V.tensor_tensor(scal[:, 4:5], scal[:, 4:5], scal[:, 8:9], Alu.max)
            V.tensor_scalar(scal[:, 4:5], scal[:, 4:5], 1.0, 0.0, Alu.max, Alu.add)
            V.tensor_tensor(scal[:, 5:6], scal[:, 2:3], scal[:, 0:1], Alu.subtract)
            V.tensor_scalar(scal[:, 8:9], scal[:, 5:6], -1.0, 0.0, Alu.mult, Alu.add)
            V.tensor_tensor(scal[:, 5:6], scal[:, 5:6], scal[:, 8:9], Alu.max)
            V.reciprocal(scal[:, 8:9], scal[:, 4:5])
            V.tensor_tensor(scal[:, 5:6], scal[:, 5:6], scal[:, 8:9], Alu.mult)
            V.tensor_scalar(scal[:, 6:7], scal[:, 3:4], 1.0,
                            -float(TOPN) if full else -TOPN / 4.0,
                            Alu.mult, Alu.add)
            V.tensor_tensor(scal[:, 6:7], scal[:, 6:7], scal[:, 5:6], Alu.mult)
            V.tensor_copy(scal[:, 0:1], scal[:, 2:3])
            V.tensor_copy(scal[:, 1:2], scal[:, 3:4])
            V.tensor_tensor(scal[:, 2:3], scal[:, 2:3], scal[:, 6:7], Alu.add)
            bcast_thr(s, scal[0:1, 2:3])
            f = count_into(s, 0, sub=not full)
            nc.scalar.copy(scal[:, 3:4], f[:])

        def f_msums(s):
            st = ST[s]
            u, acc8, thr = uh[s], st["acc8"], st["thr"]
            V.tensor_scalar(junk, u[:, 0:NW], thr[:], 0.0,
                            Alu.is_gt, Alu.bypass)
            mbufs = (poolT[:], uhTp[:, 0:NW], mxp[:, 0:NW])
            for chn, xt in enumerate(x16[s]):
                mb = mbufs[chn % 3]
                V.tensor_tensor(mb, junk, xt[:], Alu.mult)
                nc.scalar.activation(mb, mb, Act.Copy,
                                     accum_out=acc8[:, 1 + chn:2 + chn])

        def f_bandprep(s):
            st = ST[s]
            scal = st["scal"]
            V.tensor_scalar(scal[:, 7:8], scal[:, 2:3], 1.0, -BAND,
                            Alu.mult, Alu.add)
            bcast_thr(s, scal[0:1, 7:8])

        def f_bandsums(s):
            st = ST[s]
            u, acc8, thr = uh[s], st["acc8"], st["thr"]
            V.tensor_scalar(junk, u[:, 0:NW], thr[:], 0.0,
                            Alu.is_gt, Alu.bypass)
            nc.scalar.activation(poolT[:], junk, Act.Copy,
                                 accum_out=acc8[:, 4:5])
            mbufs = (poolT[:], uhTp[:, 0:NW], mxp[:, 0:NW])
            for chn, xt in enumerate(x16[s]):
                mb = mbufs[chn % 3]
                V.tensor_tensor(mb, junk, xt[:], Alu.mult)
                nc.scalar.activation(mb, mb, Act.Copy,
                                     accum_out=acc8[:, 5 + chn:6 + chn])

        def f_afold(s):
            st = ST[s]
            tps = psml.tile([1, 8], f32, name=f"totps{s}", tag=f"fold{s}")
            nc.tensor.matmul(tps[:], c_ones128[:], st["acc8"][:],
                             start=True, stop=True)
            tot = tiny.tile([1, 8], f32, name=f"tot{s}", tag=f"tot{s}")
            nc.scalar.copy(tot[:], tps[:])
            st["tot"] = tot

        def f_amath(s):
            st = ST[s]
            tot = st["tot"]
            am = tiny.tile([1, 12], f32, name=f"am{s}", tag=f"am{s}")
            V.tensor_tensor(am[:, 0:3], tot[:, 5:8], tot[:, 1:4], Alu.subtract)
            V.tensor_tensor(am[:, 11:12], tot[:, 4:5], tot[:, 0:1], Alu.subtract)
            V.tensor_scalar(am[:, 11:12], am[:, 11:12], 1.0, 0.0, Alu.max, Alu.add)
            V.reciprocal(am[:, 10:11], am[:, 11:12])
            V.tensor_tensor(am[:, 0:3], am[:, 0:3], fbcast(am[:, 10:11], 3), Alu.mult)
            V.tensor_scalar(am[:, 9:10], tot[:, 0:1], -1.0, float(TOPN),
                            Alu.mult, Alu.add)
            V.tensor_tensor(am[:, 0:3], am[:, 0:3], fbcast(am[:, 9:10], 3), Alu.mult)
            V.tensor_tensor(am[:, 0:3], am[:, 0:3], tot[:, 1:4], Alu.add)
            V.tensor_scalar(am[:, 0:3], am[:, 0:3], 1.0 / TOPN, 0.0, Alu.mult, Alu.add)
            V.tensor_scalar(am[:, 3:6], am[:, 0:3], 1.0, 1.0, Alu.mult, Alu.add)
            V.reciprocal(am[:, 3:6], am[:, 3:6])
            V.tensor_scalar(am[:, 0:3], am[:, 0:3], 0.5, 0.5, Alu.mult, Alu.add)
            V.tensor_scalar(am[:, 6:9], am[:, 0:3], -1.0, 0.5, Alu.mult, Alu.add)
            st["am"] = am

        def f_chsc(s):
            st = ST[s]
            st["chsc"] = tiny.tile([128, 9], f32, name=f"chsc{s}",
                                   tag=f"chsc{s}")
            bp = psml.tile([128, 9], f32, name=f"chps{s}", tag=f"fold{s}")
            nc.tensor.matmul(bp[:], c_ones1x[:], st["am"][0:1, 0:9],
                             start=True, stop=True)
            nc.scalar.copy(st["chsc"][:], bp[:])

        # ------------------------------------------- guidance-only box prep
        def prep_ops(s):
            yield lambda: pe_sub(sview(t_is[s]), t_guid[s])
            yield lambda: nc.scalar.activation(sview(t_ii[s]), sview(t_is[s]),
                                               Act.Square)
            hbI = [None]
            hbII = [None]
            mII = [None]

            def scanI():
                hbI[0] = rot.tile([128, NWS], f32r, name="hbI", tag="hbx")
                hbox_s(hbI[0], t_is[s])
            yield scanI
            yield lambda: vbox_s(mean_Is[s], hbI[0])

            def scanII():
                hbII[0] = rot.tile([128, NWS], f32r, name="hbII", tag="hbx")
                hbox_s(hbII[0], t_ii[s])
            yield scanII

            def vboxII():
                mII[0] = mrot.tile([128, NWS], f32, name="mII", tag="mpx")
                vbox_s(mII[0], hbII[0])
            yield vboxII

            def varrec():
                sq = sab.tile([128, NWS], f32, name="sq", tag="sab")
                nc.scalar.activation(sq[:], mean_Is[s][:], Act.Square)
                V.scalar_tensor_tensor(sq[:], mII[0][:], EPS, sq[:],
                                       Alu.add, Alu.subtract)
                V.reciprocal_approx_fast(out=rec_s[s][:], in_=sq[:])
            yield varrec

        # ---------------------------------------------------------- backend
        BK = [dict(), dict()]

        def backend_head(s):
            pe_sub(sview(t_ps[s]), uh[s], scale=-OMEGA, bias=1.0)
            V.tensor_tensor(sview(t_ip[s]), sview(t_is[s]), sview(t_ps[s]),
                            Alu.mult)
            hb_p = rot.tile([128, NWS], f32r, name="hb_p", tag="hbx")
            hbox_s(hb_p, t_ps[s])
            mean_p = mrot.tile([128, NWS], f32, name="mean_p", tag="mpx")
            vbox_s(mean_p, hb_p)
            hb_ip = rot.tile([128, NWS], f32r, name="hb_ip", tag="hbx")
            hbox_s(hb_ip, t_ip[s])
            mean_Ip = mrot.tile([128, NWS], f32, name="mean_Ip", tag="mpx")
            vbox_s(mean_Ip, hb_ip)
            BK[s]["mp"], BK[s]["mip"] = mean_p, mean_Ip

        def backend_mid(s):
            mean_p, mean_Ip = BK[s]["mp"], BK[s]["mip"]
            tmp = sab.tile([128, NWS], f32, name="tmp", tag="sab")
            V.tensor_tensor(tmp[:], mean_Is[s][:], mean_p[:], Alu.mult)
            cov = sab.tile([128, NWS], f32, name="cov", tag="sab")
            V.tensor_tensor(cov[:], mean_Ip[:], tmp[:], Alu.subtract)
            a_v = sview(t_ip[s])          # overwrite Ip (dead) with a
            V.tensor_tensor(a_v, cview(cov, WS)[:, :, :],
                            cview(rec_s[s], WS)[:, :, :], Alu.mult)
            t2 = sab.tile([128, NWS], f32, name="t2", tag="sab")
            V.tensor_tensor(cview(t2, WS)[:, :, :], a_v,
                            cview(mean_Is[s], WS)[:, :, :], Alu.mult)
            b_v = sview(t_ps[s])          # overwrite p (dead) with b
            V.tensor_tensor(b_v, cview(mean_p, WS)[:, :, :],
                            cview(t2, WS)[:, :, :], Alu.subtract)

            hba = rot.tile([128, NWS], f32r, name="hba", tag="hbx")
            hbox_s(hba, t_ip[s])
            mean_a = mrot.tile([128, NWS], f32, name="mean_a", tag="mpx")
            vbox_s(mean_a, hba)
            hbb = rot.tile([128, NWS], f32r, name="hbb", tag="hbx")
            hbox_s(hbb, t_ps[s])
            mean_b = mrot.tile([128, NWS], f32, name="mean_b", tag="mpx")
            vbox_s(mean_b, hbb)
            ma_f = mfull.tile([128, NW], bf16, name="ma_f", tag="mf")
            upsample(ma_f, mean_a)
            mb_f = mfull.tile([128, NW], bf16, name="mb_f", tag="mf")
            upsample(mb_f, mean_b)
            BK[s]["ma"], BK[s]["mb"] = ma_f, mb_f

        def backend_tail(s):
            chsc = ST[s]["chsc"]
            ma_f, mb_f = BK[s]["ma"], BK[s]["mb"]
            HW_ = NW // 2
            T16 = dout.tile([128, NW], bf16, name="T16", tag="dout")
            T_t = abt.tile([128, NW], f32, name="T_t", tag="abt")
            rT = abt.tile([128, NW], f32, name="rT", tag="abt")
            for h in (0, 1):
                sl = slice(h * HW_, (h + 1) * HW_)
                V.tensor_tensor(T16[:, sl], ma_f[:, sl], t_guid[s][:, sl],
                                Alu.mult)
                V.tensor_tensor(T16[:, sl], T16[:, sl], mb_f[:, sl], Alu.add)
                nc.scalar.copy(T_t[:, sl], T16[:, sl])
                V.reciprocal_approx_fast(out=rT[:, sl], in_=T_t[:, sl])
                if s == 0:
                    nc.scalar.copy(poolT[:, sl], rT[:, sl])
            rmul = poolT if s == 0 else rT

            for chn in range(3):
                d_t = dout.tile([128, NW], bf16, name=f"d{chn}", tag="dout")
                yv = y_ext[s, chn].rearrange("(c p) w -> p c w", p=128)
                for h in (0, 1):
                    sl = slice(h * HW_, (h + 1) * HW_)
                    nc.scalar.activation(d_t[:, sl], x16[s][chn][:, sl],
                                         Act.Identity,
                                         bias=chsc[:, 6 + chn:7 + chn],
                                         scale=0.5)
                    V.tensor_tensor(d_t[:, sl], d_t[:, sl], rmul[:, sl],
                                    Alu.mult)
                    V.tensor_scalar(d_t[:, sl], d_t[:, sl],
                                    chsc[:, chn:chn + 1], 0.0,
                                    Alu.add, Alu.add)
                    nc.gpsimd.dma_start(out=yv[:, 2 * h:2 * h + 2, :],
                                        in_=cview(d_t)[:, 2 * h:2 * h + 2, :])

        # ================================================== emission order
        f_load(0)
        f_load(1)
        load_consts()

        # dark1 for sample 0 only; sample 1's pool runs inside s0's secant
        f_chanmin_hpool(0, False)
        t_fwd(0)
        f_guid(0)
        f_hpoolT(0)
        t_back(0)

        f_secant_init(0)
        f_secant_init(1)
        fills = [lambda: f_chanmin_hpool(1, False),
                 lambda: t_fwd(1),
                 lambda: f_guid(1),
                 lambda: f_hpoolT(1),
                 lambda: t_back(1)]
        fills += list(prep_ops(0)) + list(prep_ops(1))
        pi = 0

        def drain_prep(n=1):
            nonlocal pi
            for _ in range(n):
                if pi < len(fills):
                    fills[pi]()
                    pi += 1

        # sample-0 secant units, with sample-1's pool + preps as fill;
        # sample-1 secant lags by 3 units (its pooled dark is ready then)
        q0 = [lambda w=w: f_count0(0, w) for w in (0, 1)]
        q0 += [lambda r=r: f_secant_round(0, r) for r in range(SECANT_ROUNDS)]
        q1 = [lambda w=w: f_count0(1, w) for w in (0, 1)]
        q1 += [lambda r=r: f_secant_round(1, r) for r in range(SECANT_ROUNDS)]
        q0[0]()
        drain_prep(3)
        q0[1]()
        drain_prep(2)
        for i in range(2, len(q0)):
            q1[i - 2]()
            q0[i]()
            drain_prep()
        f_msums(0)
        q1[-2]()
        f_bandprep(0)
        q1[-1]()
        drain_prep()
        f_bandsums(0)
        f_msums(1)
        f_bandprep(1)
        drain_prep()
        f_afold(0)
        f_bandsums(1)
        drain_prep(len(fills))
        f_afold(1)
        f_amath(0)
        f_amath(1)
        f_chsc(0)
        f_chsc(1)

        # dark2 phase with backend_head(0) interleaved after sample 0's pool
        f_chanmin_hpool(0, True)
        t_fwd(0)
        f_chanmin_hpool(1, True)
        f_hpoolT(0)
        t_back(0)
        backend_head(0)
        t_fwd(1)
        f_hpoolT(1)
        t_back(1)
        backend_mid(0)
        backend_head(1)
        backend_tail(0)
        backend_mid(1)
        backend_tail(1)

    nc.compile()
    return nc


def _get_program():
    if "nc" not in _CACHE:
        _CACHE["nc"] = _build()
    return _CACHE["nc"]


def kernel(x: np.ndarray) -> np.ndarray:
    from concourse.bass_utils import run_bass_kernel_spmd
    x = np.ascontiguousarray(np.asarray(x, dtype=np.float32))
    assert x.shape == (16, 3, H, W), x.shape
    nc = _get_program()
    consts = _host_consts()
    in_maps = [{"x": x[2 * i:2 * i + 2], **consts} for i in range(8)]
    res = run_bass_kernel_spmd(nc, in_maps, list(range(8)))
    out = np.concatenate([res.results[i]["y"] for i in range(8)], axis=0)
    return out.astype(np.float32)


# revision 64
# speedup vs baseline: 1.0482x; 1.0131x over previous
"""Dark-Channel-Prior dehazing (DCPGenerator) Trainium2 Bass kernel, v9.

v8 -> v9: the guided filter runs as a fast-guided-filter at 2x subsample
(256x256): all six box filters (I, II, p, Ip, a, b), the cov/var/a/b
math, and the vbox matmuls operate on 1/4 the pixels with radius-20
bands; mean_a/mean_b are bilinearly upsampled (PE matmuls for rows, DVE
for columns) and T = mean_a*I + mean_b is applied at full resolution.
Subsampling of guid / pooled-dark runs on the PE with selection
matrices.  Dark channel, top-k secant, and A estimation stay full-res.
"""
import numpy as np
from contextlib import ExitStack

H = 512
W = 512
NCHUNK = 4
CW = 512
NW = NCHUNK * CW            # 2048
PADW = 526                  # 7 | 512 | 7
WIN_PAD = 7
RADIUS = 40
# sub-grid (fast guided filter, s=2)
HS = 256
WS = 256
NCS = 2
RS = 20
LEAD = 24                   # leading zeros in sub scan layout (>=RS+1)
SEG = 300                   # WS + 44-zero gap (>= 2*RS+1)
SCN_W = LEAD + NCS * SEG    # 624
NWS = NCS * WS              # 512
EPS = 1e-3
OMEGA = 0.95
TOPN = int(0.01 * H * W)    # 2621
T0 = 0.0055
T1 = 0.0085
BAND = 2e-4
SECANT_ROUNDS = 5

_CACHE = {}


def _host_consts():
    # full-res H-direction box weights are no longer needed; sub-grid ones:
    i = np.arange(HS)
    n1s = np.minimum(i + RS, HS - 1) - np.maximum(i - RS, 0) + 1
    inv_ns = (1.0 / n1s).astype(np.float32)
    k = np.arange(128)[:, None]
    p = np.arange(128)[None, :]
    bands = (np.abs(k - p) <= RS).astype(np.float32)
    bus = (k >= p + 128 - RS).astype(np.float32) / 41.0 / 41.0
    bds = (k <= p - (128 - RS)).astype(np.float32) / 41.0 / 41.0
    bm0s = bands * inv_ns[0:128][None, :] / 41.0
    bm1s = bands * inv_ns[128:256][None, :] / 41.0
    fix40 = np.concatenate([41.0 * inv_ns[0:RS], 41.0 * inv_ns[WS - RS:]])
    fixs = np.tile(fix40[None, :], (128, NCS)).copy()        # [128, 80]
    ident = np.eye(128, dtype=np.float32)
    # row-subsample selection: out q <- full partition 2q (two half matrices)
    selA = np.zeros((128, 128), np.float32)
    selB = np.zeros((128, 128), np.float32)
    for q in range(64):
        selA[2 * q, q] = 1.0
    for q in range(64, 128):
        selB[2 * (q - 64), q] = 1.0
    # row-upsample (bilinear, sub sample i at full row 2i)
    U = {}
    for c in range(NCHUNK):
        for q in range(128):
            r = 128 * c + q
            if r % 2 == 0:
                pairs = [(r // 2, 1.0)]
            else:
                i0 = (r - 1) // 2
                i1 = min(i0 + 1, HS - 1)
                pairs = [(i0, 0.5), (i1, 0.5)] if i1 != i0 else [(i0, 1.0)]
            for i_, wgt in pairs:
                sc, pp_ = divmod(i_, 128)
                U.setdefault((c, sc), np.zeros((128, 128), np.float32))[
                    pp_, q] += wgt
    return {"bm0s": bm0s, "bm1s": bm1s, "bus": bus, "bds": bds,
            "fixs": fixs, "ident": ident, "selA": selA, "selB": selB,
            "u00": U[(0, 0)], "u10": U[(1, 0)], "u11": U[(1, 1)],
            "u21": U[(2, 1)], "u31": U[(3, 1)]}


def _build():
    import concourse.bacc as bacc
    import concourse.tile as tile
    import concourse.bass as bass
    from concourse import mybir

    f32 = mybir.dt.float32
    f32r = mybir.dt.float32r
    bf16 = mybir.dt.bfloat16
    Alu = mybir.AluOpType
    Act = mybir.ActivationFunctionType

    nc = bacc.Bacc("TRN2", target_bir_lowering=False, debug=False, num_devices=8)
    V = nc.vector
    G = nc.gpsimd

    x_ext = nc.dram_tensor("x", [2, 3, H, W], f32, kind="ExternalInput").ap()
    c128_names = ("bm0s", "bm1s", "bus", "bds", "ident", "selA", "selB",
                  "u00", "u10", "u11", "u21", "u31")
    c128_exts = {nm: nc.dram_tensor(nm, [128, 128], f32, kind="ExternalInput").ap()
                 for nm in c128_names}
    fixs_ext = nc.dram_tensor("fixs", [128, NCS * 2 * RS], f32,
                              kind="ExternalInput").ap()
    y_ext = nc.dram_tensor("y", [2, 3, H, W], f32, kind="ExternalOutput").ap()

    def cview(t, width=CW):
        return t.rearrange("p (c w) -> p c w", w=width)

    def fbcast(ap_col, n):
        return bass.AP(tensor=ap_col.tensor, offset=ap_col.offset,
                       ap=[list(p) for p in ap_col.ap[:-1]] + [[0, n]])

    def segview(t, off, c0=0, nch=NCS):
        """[128, nch, WS] view into a [128, SCN_W] sub tile."""
        base = t[:]
        return bass.AP(tensor=base.tensor, offset=base.offset + off + c0 * SEG,
                       ap=[list(base.ap[0]), [SEG, nch], [1, WS]])

    def sview(t):
        return segview(t, LEAD)

    with ExitStack() as ctx:
        tc = ctx.enter_context(tile.TileContext(nc))

        cpool = ctx.enter_context(tc.tile_pool(name="cpool", bufs=1))
        srcp = ctx.enter_context(tc.tile_pool(name="srcp", bufs=1))
        scn = ctx.enter_context(tc.tile_pool(name="scn", bufs=1))
        pp = ctx.enter_context(tc.tile_pool(name="pp", bufs=1))
        cump = ctx.enter_context(tc.tile_pool(name="cump", bufs=2))
        boxes = ctx.enter_context(tc.tile_pool(name="boxes", bufs=1))
        rot = ctx.enter_context(tc.tile_pool(name="rot", bufs=2))
        mrot = ctx.enter_context(tc.tile_pool(name="mrot", bufs=4))
        abt = ctx.enter_context(tc.tile_pool(name="abt", bufs=3))
        sab = ctx.enter_context(tc.tile_pool(name="sab", bufs=3))
        dout = ctx.enter_context(tc.tile_pool(name="dout", bufs=2))
        mfull = ctx.enter_context(tc.tile_pool(name="mfull", bufs=2))
        tiny = ctx.enter_context(tc.tile_pool(name="tiny", bufs=1))
        pbig = ctx.enter_context(tc.tile_pool(name="pbig", bufs=1, space="PSUM"))
        pmid = ctx.enter_context(tc.tile_pool(name="pmid", bufs=2, space="PSUM"))
        psml = ctx.enter_context(tc.tile_pool(name="psml", bufs=1, space="PSUM"))

        # ------------------------------------- constants (loaded after x DMAs)
        cbf = {}
        stage = cpool.tile([128, 128], f32, name="s_band")
        for nm in ("bm0s", "bm1s", "bus", "bds"):
            cbf[nm] = cpool.tile([128, 128], f32r, name=f"c_{nm}")
        for nm in ("ident", "selA", "selB", "u00", "u10", "u11", "u21", "u31"):
            cbf[nm] = cpool.tile([128, 128], bf16, name=f"c_{nm}")
        c_fixs = cpool.tile([128, NCS * 2 * RS], f32, name="c_fixs")
        c_ones128 = cpool.tile([128, 1], f32, name="c_ones128")
        c_ones1x = cpool.tile([1, 128], f32, name="c_ones1x")

        def load_consts():
            for nm in ("bm0s", "bm1s", "bus", "bds", "ident", "selA", "selB",
                       "u00", "u10", "u11", "u21", "u31"):
                nc.sync.dma_start(out=stage[:], in_=c128_exts[nm][:])
                nc.scalar.copy(cbf[nm][:], stage[:])
            nc.sync.dma_start(out=c_fixs[:], in_=fixs_ext[:])
            V.memset(c_ones128[:], 1.0)
            V.memset(c_ones1x[:], 1.0)

        # --------------------------------------------------- persistent tiles
        x16 = [[srcp.tile([128, NW], bf16, name=f"x16_{s}_{c}")
                for c in range(3)] for s in range(2)]
        t_guid = [srcp.tile([128, NW], bf16, name=f"guid{s}") for s in range(2)]
        # sub-grid scan-layout sources (f32): I, p, Ip, II, a, b per sample
        t_is = [scn.tile([128, SCN_W], f32, name=f"is{s}") for s in range(2)]
        t_ps = [scn.tile([128, SCN_W], f32, name=f"ps{s}") for s in range(2)]
        t_ip = [scn.tile([128, SCN_W], f32, name=f"ip{s}") for s in range(2)]
        t_ii = [scn.tile([128, SCN_W], f32, name=f"ii{s}") for s in range(2)]
        mxp = pp.tile([128, NCHUNK * PADW], bf16, name="mxp")
        w1 = pp.tile([128, NCHUNK * PADW], bf16, name="w1")
        uhTp = pp.tile([128, NCHUNK * PADW], bf16, name="uhTp")
        poolT = pp.tile([128, NW], bf16, name="poolT")
        uh = [pp.tile([128, NW], bf16, name=f"uh{s}") for s in range(2)]
        mean_Is = [boxes.tile([128, NWS], f32, name=f"meanIs{s}")
                   for s in range(2)]
        rec_s = [boxes.tile([128, NWS], f32, name=f"recs{s}") for s in range(2)]

        junkt = pp.tile([128, NW], bf16, name="junkt")
        junk = junkt[:, 0:NW]
        junk_c = junk.rearrange("p (c w) -> p c w", w=CW)

        # zero the sub scan-layout gaps once
        for t in (t_is[0], t_is[1], t_ps[0], t_ps[1], t_ip[0], t_ip[1],
                  t_ii[0], t_ii[1]):
            V.memset(t[:, 0:LEAD], 0.0)
            for c in range(NCS):
                V.memset(t[:, LEAD + c * SEG + WS: LEAD + (c + 1) * SEG], 0.0)

        # ---------------------------------------------------------- helpers
        def interior(t):
            return cview(t, PADW)[:, :, WIN_PAD:WIN_PAD + CW]

        def memset_pads(t, eng):
            v = cview(t, PADW)
            for c in range(NCHUNK):
                eng.memset(v[:, c, 0:WIN_PAD], 1.0)
                eng.memset(v[:, c, PADW - WIN_PAD:PADW], 1.0)

        def hpool(dst, padded, scratch):
            a = cview(padded, PADW)
            b = cview(scratch, PADW)
            d = cview(dst)
            V.tensor_tensor(b[:, :, 0:525], a[:, :, 0:525], a[:, :, 1:526], Alu.min)
            V.tensor_tensor(a[:, :, 0:523], b[:, :, 0:523], b[:, :, 2:525], Alu.min)
            V.tensor_tensor(b[:, :, 0:519], a[:, :, 0:519], a[:, :, 4:523], Alu.min)
            V.tensor_tensor(d[:, 0:NCHUNK, :], b[:, :, 0:512], b[:, :, 7:519],
                            Alu.min)

        def transpose_blocks(dst_ap, src_flat, on_dve=False):
            sv = cview(src_flat)
            pt = pbig.tile([128, NW], bf16, name="pt", tag="ptp")
            for co in range(NCHUNK):
                for ci in range(NCHUNK):
                    nc.tensor.transpose(
                        pt[:, co * CW + ci * 128: co * CW + (ci + 1) * 128],
                        sv[:, ci, co * 128:(co + 1) * 128], cbf["ident"][:])
            if on_dve:
                V.tensor_copy(dst_ap, cview(pt)[:, :, :])
            else:
                nc.scalar.copy(dst_ap, cview(pt)[:, :, :])

        def t_fwd(s, on_dve=False):
            memset_pads(uhTp, G)
            iv = cview(uhTp, PADW)
            transpose_blocks(iv[:, :, WIN_PAD:WIN_PAD + CW], uh[s], on_dve)

        def t_back(s, on_dve=False):
            transpose_blocks(cview(uh[s])[:, :, :], poolT, on_dve)

        # ------------------------------------------------ sub-grid helpers
        def pe_sub(dst_seg_ap, src_full, scale=1.0, bias=0.0):
            """dst (sub scan-layout data view) <- src_full[::2,::2]*scale+bias."""
            sv = cview(src_full)
            ps = pmid.tile([128, NWS], f32, name="subps", tag="pmid")
            for cs in range(NCS):
                psc = ps[:, cs * WS:(cs + 1) * WS]
                nc.tensor.matmul(psc, cbf["selA"][:],
                                 sv[:, 2 * cs, 0:CW:2], start=True, stop=False)
                nc.tensor.matmul(psc, cbf["selB"][:],
                                 sv[:, 2 * cs + 1, 0:CW:2], start=False,
                                 stop=True)
            if scale == 1.0 and bias == 0.0:
                nc.scalar.copy(dst_seg_ap, cview(ps, WS)[:, :, :])
            else:
                nc.scalar.activation(dst_seg_ap, cview(ps, WS)[:, :, :],
                                     Act.Copy, bias=bias, scale=scale)

        def pe_sub_T(dst_seg_ap, scale, bias):
            """subsample from poolT (transposed pooled image) + re-transpose:
            avoids the full-res back-transpose of the dark2 pool."""
            pv = cview(poolT)
            ps = pmid.tile([128, NWS], f32, name="subTps", tag="pmid")
            for cs in range(NCS):
                psc = ps[:, cs * WS:(cs + 1) * WS]
                nc.tensor.matmul(psc, cbf["selA"][:],
                                 pv[:, 2 * cs, 0:CW:2], start=True, stop=False)
                nc.tensor.matmul(psc, cbf["selB"][:],
                                 pv[:, 2 * cs + 1, 0:CW:2], start=False,
                                 stop=True)
            tT = dout.tile([128, NWS], bf16, name="tT", tag="dout")
            nc.scalar.copy(tT[:], ps[:])
            ps2 = pmid.tile([128, NWS], bf16, name="subT2", tag="pmid")
            tv = cview(tT, WS)
            for ch in range(NCS):
                for cw in range(NCS):
                    nc.tensor.transpose(
                        ps2[:, ch * WS + cw * 128: ch * WS + (cw + 1) * 128],
                        tv[:, cw, ch * 128:(ch + 1) * 128], cbf["ident"][:])
            nc.scalar.activation(dst_seg_ap, cview(ps2, WS)[:, :, :],
                                 Act.Copy, bias=bias, scale=scale)

        def hbox_s(hb_t, src_t):
            cum = cump.tile([128, SCN_W], f32, name="cum", tag="cum")
            for c in range(NCS):
                V.tensor_tensor_scan(cum[:, c * SEG:(c + 1) * SEG],
                                     src_t[:, c * SEG:(c + 1) * SEG],
                                     fbcast(c_ones128[:, 0:1], SEG), 0.0,
                                     Alu.add, Alu.bypass)
            V.tensor_tensor(cview(hb_t, WS)[:, :, :],
                            segview(cum, LEAD + RS),
                            segview(cum, LEAD - RS - 1), Alu.subtract)

        def vbox_s(dst, src):
            sv = cview(src, WS)
            ps = pmid.tile([128, NWS], f32, name="vps", tag="pmid")
            r0 = ps[:, 0:WS]
            r1 = ps[:, WS:NWS]
            nc.tensor.matmul(r0, cbf["bm0s"][:], sv[:, 0, :], start=True,
                             stop=False)
            nc.tensor.matmul(r0, cbf["bds"][:], sv[:, 1, :], start=False,
                             stop=True)
            nc.tensor.matmul(r1, cbf["bm1s"][:], sv[:, 1, :], start=True,
                             stop=False)
            nc.tensor.matmul(r1, cbf["bus"][:], sv[:, 0, :], start=False,
                             stop=True)
            nc.scalar.copy(dst[:], ps[:])
            db = dst[:]
            edges = bass.AP(tensor=db.tensor, offset=db.offset,
                            ap=[list(db.ap[0]), [WS, NCS],
                                [WS - RS, 2], [1, RS]])
            fb = c_fixs[:]
            fv = bass.AP(tensor=fb.tensor, offset=fb.offset,
                         ap=[list(fb.ap[0]), [2 * RS, NCS], [RS, 2], [1, RS]])
            V.tensor_tensor(edges, edges, fv, Alu.mult)

        def upsample(dst_full_bf16, src_sub):
            """bilinear 2x upsample [128, 2x256] f32 -> [128, 4x512] bf16."""
            wide = mrot.tile([128, NCS * CW], bf16, name="wide", tag="wide",
                             bufs=2)
            wv = cview(wide)
            sv = cview(src_sub, WS)
            # W-upsample at sub rows
            wide_e = bass.AP(tensor=wv.tensor, offset=wv.offset,
                             ap=[list(wv.ap[0]), [CW, NCS], [2, WS]])
            V.tensor_copy(wide_e, sv[:, :, :])
            wide_o = bass.AP(tensor=wv.tensor, offset=wv.offset + 1,
                             ap=[list(wv.ap[0]), [CW, NCS], [2, WS - 1]])
            V.tensor_tensor(wide_o, sv[:, :, 0:WS - 1], sv[:, :, 1:WS], Alu.add)
            V.tensor_scalar(wide_o, wide_o, 0.5, 0.0, Alu.mult, Alu.add)
            lastc = bass.AP(tensor=wv.tensor, offset=wv.offset + CW - 1,
                            ap=[list(wv.ap[0]), [CW, NCS], [1, 1]])
            V.tensor_copy(lastc, sv[:, :, WS - 1:WS])
            # H-upsample via PE
            ps = pbig.tile([128, NW], f32, name="ups", tag="ptp")
            for c, mats in enumerate((
                    (("u00", 0),), (("u10", 0), ("u11", 1)),
                    (("u21", 1),), (("u31", 1),))):
                psc = ps[:, c * CW:(c + 1) * CW]
                for i, (nm, sc) in enumerate(mats):
                    nc.tensor.matmul(psc, cbf[nm][:], wv[:, sc, :],
                                     start=(i == 0), stop=(i == len(mats) - 1))
            nc.scalar.copy(dst_full_bf16[:], ps[:])

        # ---------------------------------------------- per-sample frontend
        ST = [dict(), dict()]

        def f_load(s):
            for chn in range(3):
                src = x_ext[s, chn].rearrange("(c p) w -> p c w", p=128)
                if s == 0 and chn >= 1:
                    # spread sample 0 across all three DMA paths so its three
                    # channels land concurrently (SWDGE cast alone is slow)
                    ring = nc.sync if chn == 1 else nc.scalar
                    stg = abt.tile([128, NW], f32, name=f"xstg{chn}",
                                   tag="abt")
                    ring.dma_start(out=cview(stg)[:, :, :], in_=src)
                    nc.scalar.copy(x16[s][chn][:], stg[:])
                else:
                    nc.gpsimd.dma_start(out=cview(x16[s][chn])[:, :, :], in_=src)

        def f_guid(s):
            gt = t_guid[s]
            tg = dout.tile([128, NW], bf16, name=f"gt{s}", tag="dout")
            tb = dout.tile([128, NW], bf16, name=f"bt{s}", tag="dout")
            nc.scalar.activation(gt[:], x16[s][0][:], Act.Copy,
                                 bias=0.5, scale=0.14945)
            nc.scalar.activation(tg[:], x16[s][1][:], Act.Copy,
                                 bias=0.0, scale=0.2935)
            nc.scalar.activation(tb[:], x16[s][2][:], Act.Copy,
                                 bias=0.0, scale=0.057)
            V.tensor_tensor(gt[:], gt[:], tg[:], Alu.add)
            V.tensor_tensor(gt[:], gt[:], tb[:], Alu.add)

        def f_chanmin_hpool(s, second):
            memset_pads(mxp, G)
            if not second:
                a0, a1, a2 = x16[s]
                V.tensor_tensor(interior(mxp), cview(a0)[:, :, :],
                                cview(a1)[:, :, :], Alu.min)
                V.tensor_tensor(interior(mxp), interior(mxp),
                                cview(a2)[:, :, :], Alu.min)
            else:
                chsc = ST[s]["chsc"]
                ytmp = junk
                nc.scalar.activation(interior(mxp), x16[s][0][:], Act.Identity,
                                     bias=chsc[:, 3:4], scale=chsc[:, 3:4])
                nc.scalar.activation(ytmp, x16[s][1][:], Act.Identity,
                                     bias=chsc[:, 4:5], scale=chsc[:, 4:5])
                V.tensor_tensor(interior(mxp), interior(mxp), junk_c, Alu.min)
                nc.scalar.activation(ytmp, x16[s][2][:], Act.Identity,
                                     bias=chsc[:, 5:6], scale=chsc[:, 5:6])
                V.tensor_tensor(interior(mxp), interior(mxp), junk_c, Alu.min)
            hpool(uh[s], mxp, w1)

        def f_hpoolT(s):
            hpool(poolT, uhTp, w1)

        def dark_phase(second):
            f_chanmin_hpool(0, second)
            t_fwd(0)
            f_chanmin_hpool(1, second)
            if not second:
                f_guid(0)
            f_hpoolT(0)
            t_back(0)
            t_fwd(1)
            if not second:
                f_guid(1)
            f_hpoolT(1)
            t_back(1)

        # ------------------------------------------------------- secant/topk
        def f_secant_init(s):
            st = ST[s]
            st["acc8"] = tiny.tile([128, 8], f32, name=f"acc8{s}", tag=f"acc8{s}")
            V.memset(st["acc8"][:], 0.0)
            st["thr"] = tiny.tile([128, 1], f32, name=f"thr{s}", tag=f"thr{s}")
            st["scal"] = tiny.tile([1, 16], f32, name=f"scal{s}", tag=f"scal{s}")
            V.memset(st["scal"][:], 0.0)
            V.memset(st["scal"][:, 0:1], T0)
            V.memset(st["scal"][:, 2:3], T1)

        def count_into(s, col, sub=False):
            st = ST[s]
            u, acc8, thr = uh[s], st["acc8"], st["thr"]
            uv = cview(u)
            if sub:
                V.tensor_scalar(junk_c[:, 0:2, 0:256],
                                uv[:, 0:NCHUNK:2, 0:CW:2], thr[:], 0.0,
                                Alu.is_gt, Alu.add,
                                accum_out=acc8[:, col:col + 1])
            else:
                V.tensor_scalar(junk, u[:, 0:NW], thr[:], 0.0,
                                Alu.is_gt, Alu.add,
                                accum_out=acc8[:, col:col + 1])
            fps = psml.tile([1, 1], f32, name=f"fold{s}", tag=f"fold{s}")
            nc.tensor.matmul(fps[:], c_ones128[:], acc8[:, col:col + 1],
                             start=True, stop=True)
            return fps

        def bcast_thr(s, src_col):
            st = ST[s]
            bp = psml.tile([128, 1], f32, name=f"thrps{s}", tag=f"fold{s}")
            nc.tensor.matmul(bp[:], c_ones1x[:], src_col, start=True, stop=True)
            nc.scalar.copy(st["thr"][:], bp[:])

        def f_count0(s, which):
            scal = ST[s]["scal"]
            col = 0 if which == 0 else 2
            bcast_thr(s, scal[0:1, col:col + 1])
            f = count_into(s, 0, sub=True)
            nc.scalar.copy(scal[:, col + 1:col + 2], f[:])

        def f_secant_round(s, rnd):
            scal = ST[s]["scal"]
            full = rnd >= SECANT_ROUNDS - 2
            if rnd == SECANT_ROUNDS - 2:
                V.tensor_scalar(scal[:, 1:2], scal[:, 1:2], 4.0, 0.0,
                                Alu.mult, Alu.add)
                V.tensor_scalar(scal[:, 3:4], scal[:, 3:4], 4.0, 0.0,
                                Alu.mult, Alu.add)
            V.tensor_tensor(scal[:, 4:5], scal[:, 3:4], scal[:, 1:2], Alu.subtract)
            V.tensor_scalar(scal[:, 8:9], scal[:, 4:5], -1.0, 0.0, Alu.mult, Alu.add)
            V.tensor_tensor(scal[:, 4:5], scal[:, 4:5], scal[:, 8:9], Alu.max)
            V.tensor_scalar(scal[:, 4:5], scal[:, 4:5], 1.0, 0.0, Alu.max, Alu.add)
            V.tensor_tensor(scal[:, 5:6], scal[:, 2:3], scal[:, 0:1], Alu.subtract)
            V.tensor_scalar(scal[:, 8:9], scal[:, 5:6], -1.0, 0.0, Alu.mult, Alu.add)
            V.tensor_tensor(scal[:, 5:6], scal[:, 5:6], scal[:, 8:9], Alu.max)
            V.reciprocal(scal[:, 8:9], scal[:, 4:5])
            V.tensor_tensor(scal[:, 5:6], scal[:, 5:6], scal[:, 8:9], Alu.mult)
            V.tensor_scalar(scal[:, 6:7], scal[:, 3:4], 1.0,
                            -float(TOPN) if full else -TOPN / 4.0,
                            Alu.mult, Alu.add)
            V.tensor_tensor(scal[:, 6:7], scal[:, 6:7], scal[:, 5:6], Alu.mult)
            V.tensor_copy(scal[:, 0:1], scal[:, 2:3])
            V.tensor_copy(scal[:, 1:2], scal[:, 3:4])
            V.tensor_tensor(scal[:, 2:3], scal[:, 2:3], scal[:, 6:7], Alu.add)
            bcast_thr(s, scal[0:1, 2:3])
            f = count_into(s, 0, sub=not full)
            nc.scalar.copy(scal[:, 3:4], f[:])

        def f_msums(s):
            st = ST[s]
            u, acc8, thr = uh[s], st["acc8"], st["thr"]
            V.tensor_scalar(junk, u[:, 0:NW], thr[:], 0.0,
                            Alu.is_gt, Alu.bypass)
            mbufs = (poolT[:], uhTp[:, 0:NW], mxp[:, 0:NW])
            for chn, xt in enumerate(x16[s]):
                if chn == 0:
                    # one channel per group stays on DVE: shortens the
                    # ACT accumulation chain that gates afold
                    V.scalar_tensor_tensor(mbufs[0], u[:, 0:NW], thr[:],
                                           xt[:], Alu.is_gt, Alu.mult,
                                           accum_out=acc8[:, 1:2])
                    continue
                mb = mbufs[chn % 3]
                V.tensor_tensor(mb, junk, xt[:], Alu.mult)
                nc.scalar.activation(mb, mb, Act.Copy,
                                     accum_out=acc8[:, 1 + chn:2 + chn])

        def f_bandprep(s):
            st = ST[s]
            scal = st["scal"]
            V.tensor_scalar(scal[:, 7:8], scal[:, 2:3], 1.0, -BAND,
                            Alu.mult, Alu.add)
            bcast_thr(s, scal[0:1, 7:8])

        def f_bandsums(s):
            st = ST[s]
            u, acc8, thr = uh[s], st["acc8"], st["thr"]
            V.tensor_scalar(junk, u[:, 0:NW], thr[:], 0.0,
                            Alu.is_gt, Alu.bypass)
            nc.scalar.activation(poolT[:], junk, Act.Copy,
                                 accum_out=acc8[:, 4:5])
            mbufs = (poolT[:], uhTp[:, 0:NW], mxp[:, 0:NW])
            for chn, xt in enumerate(x16[s]):
                if chn == 0:
                    V.scalar_tensor_tensor(mbufs[0], u[:, 0:NW], thr[:],
                                           xt[:], Alu.is_gt, Alu.mult,
                                           accum_out=acc8[:, 5:6])
                    continue
                mb = mbufs[chn % 3]
                V.tensor_tensor(mb, junk, xt[:], Alu.mult)
                nc.scalar.activation(mb, mb, Act.Copy,
                                     accum_out=acc8[:, 5 + chn:6 + chn])

        def f_afold(s):
            st = ST[s]
            tps = psml.tile([1, 8], f32, name=f"totps{s}", tag=f"fold{s}")
            nc.tensor.matmul(tps[:], c_ones128[:], st["acc8"][:],
                             start=True, stop=True)
            tot = tiny.tile([1, 8], f32, name=f"tot{s}", tag=f"tot{s}")
            nc.scalar.copy(tot[:], tps[:])
            st["tot"] = tot

        def f_amath(s):
            st = ST[s]
            tot = st["tot"]
            am = tiny.tile([1, 12], f32, name=f"am{s}", tag=f"am{s}")
            V.tensor_tensor(am[:, 0:3], tot[:, 5:8], tot[:, 1:4], Alu.subtract)
            V.tensor_tensor(am[:, 11:12], tot[:, 4:5], tot[:, 0:1], Alu.subtract)
            V.tensor_scalar(am[:, 11:12], am[:, 11:12], 1.0, 0.0, Alu.max, Alu.add)
            V.reciprocal(am[:, 10:11], am[:, 11:12])
            V.tensor_tensor(am[:, 0:3], am[:, 0:3], fbcast(am[:, 10:11], 3), Alu.mult)
            V.tensor_scalar(am[:, 9:10], tot[:, 0:1], -1.0, float(TOPN),
                            Alu.mult, Alu.add)
            V.tensor_tensor(am[:, 0:3], am[:, 0:3], fbcast(am[:, 9:10], 3), Alu.mult)
            V.tensor_tensor(am[:, 0:3], am[:, 0:3], tot[:, 1:4], Alu.add)
            V.tensor_scalar(am[:, 0:3], am[:, 0:3], 1.0 / TOPN, 0.0, Alu.mult, Alu.add)
            V.tensor_scalar(am[:, 3:6], am[:, 0:3], 1.0, 1.0, Alu.mult, Alu.add)
            V.reciprocal(am[:, 3:6], am[:, 3:6])
            V.tensor_scalar(am[:, 0:3], am[:, 0:3], 0.5, 0.5, Alu.mult, Alu.add)
            V.tensor_scalar(am[:, 6:9], am[:, 0:3], -1.0, 0.5, Alu.mult, Alu.add)
            st["am"] = am

        def f_chsc(s):
            st = ST[s]
            st["chsc"] = tiny.tile([128, 9], f32, name=f"chsc{s}",
                                   tag=f"chsc{s}")
            bp = psml.tile([128, 9], f32, name=f"chps{s}", tag=f"fold{s}")
            nc.tensor.matmul(bp[:], c_ones1x[:], st["am"][0:1, 0:9],
                             start=True, stop=True)
            nc.scalar.copy(st["chsc"][:], bp[:])

        # ------------------------------------------- guidance-only box prep
        def prep_ops(s):
            yield lambda: pe_sub(sview(t_is[s]), t_guid[s])
            yield lambda: nc.scalar.activation(sview(t_ii[s]), sview(t_is[s]),
                                               Act.Square)
            hbI = [None]
            hbII = [None]
            mII = [None]

            def scanI():
                hbI[0] = rot.tile([128, NWS], f32r, name="hbI", tag="hbx")
                hbox_s(hbI[0], t_is[s])
            yield scanI
            yield lambda: vbox_s(mean_Is[s], hbI[0])

            def scanII():
                hbII[0] = rot.tile([128, NWS], f32r, name="hbII", tag="hbx")
                hbox_s(hbII[0], t_ii[s])
            yield scanII

            def vboxII():
                mII[0] = mrot.tile([128, NWS], f32, name="mII", tag="mpx")
                vbox_s(mII[0], hbII[0])
            yield vboxII

            def varrec():
                sq = sab.tile([128, NWS], f32, name="sq", tag="sab")
                nc.scalar.activation(sq[:], mean_Is[s][:], Act.Square)
                V.scalar_tensor_tensor(sq[:], mII[0][:], EPS, sq[:],
                                       Alu.add, Alu.subtract)
                V.reciprocal_approx_fast(out=rec_s[s][:], in_=sq[:])
            yield varrec

        # ---------------------------------------------------------- backend
        BK = [dict(), dict()]

        def backend_head(s):
            pe_sub(sview(t_ps[s]), uh[s], scale=-OMEGA, bias=1.0)
            V.tensor_tensor(sview(t_ip[s]), sview(t_is[s]), sview(t_ps[s]),
                            Alu.mult)
            hb_p = rot.tile([128, NWS], f32r, name="hb_p", tag="hbx")
            hbox_s(hb_p, t_ps[s])
            mean_p = mrot.tile([128, NWS], f32, name="mean_p", tag="mpx")
            vbox_s(mean_p, hb_p)
            hb_ip = rot.tile([128, NWS], f32r, name="hb_ip", tag="hbx")
            hbox_s(hb_ip, t_ip[s])
            mean_Ip = mrot.tile([128, NWS], f32, name="mean_Ip", tag="mpx")
            vbox_s(mean_Ip, hb_ip)
            BK[s]["mp"], BK[s]["mip"] = mean_p, mean_Ip

        def backend_mid(s):
            mean_p, mean_Ip = BK[s]["mp"], BK[s]["mip"]
            tmp = sab.tile([128, NWS], f32, name="tmp", tag="sab")
            V.tensor_tensor(tmp[:], mean_Is[s][:], mean_p[:], Alu.mult)
            cov = sab.tile([128, NWS], f32, name="cov", tag="sab")
            V.tensor_tensor(cov[:], mean_Ip[:], tmp[:], Alu.subtract)
            a_v = sview(t_ip[s])          # overwrite Ip (dead) with a
            V.tensor_tensor(a_v, cview(cov, WS)[:, :, :],
                            cview(rec_s[s], WS)[:, :, :], Alu.mult)
            t2 = sab.tile([128, NWS], f32, name="t2", tag="sab")
            V.tensor_tensor(cview(t2, WS)[:, :, :], a_v,
                            cview(mean_Is[s], WS)[:, :, :], Alu.mult)
            b_v = sview(t_ps[s])          # overwrite p (dead) with b
            V.tensor_tensor(b_v, cview(mean_p, WS)[:, :, :],
                            cview(t2, WS)[:, :, :], Alu.subtract)

            hba = rot.tile([128, NWS], f32r, name="hba", tag="hbx")
            hbox_s(hba, t_ip[s])
            mean_a = mrot.tile([128, NWS], f32, name="mean_a", tag="mpx")
            vbox_s(mean_a, hba)
            hbb = rot.tile([128, NWS], f32r, name="hbb", tag="hbx")
            hbox_s(hbb, t_ps[s])
            mean_b = mrot.tile([128, NWS], f32, name="mean_b", tag="mpx")
            vbox_s(mean_b, hbb)
            ma_f = mfull.tile([128, NW], bf16, name="ma_f", tag="mf")
            upsample(ma_f, mean_a)
            mb_f = mfull.tile([128, NW], bf16, name="mb_f", tag="mf")
            upsample(mb_f, mean_b)
            BK[s]["ma"], BK[s]["mb"] = ma_f, mb_f

        def backend_tail(s):
            chsc = ST[s]["chsc"]
            ma_f, mb_f = BK[s]["ma"], BK[s]["mb"]
            HW_ = NW // 2
            T16 = dout.tile([128, NW], bf16, name="T16", tag="dout")
            T_t = abt.tile([128, NW], f32, name="T_t", tag="abt")
            rT = abt.tile([128, NW], f32, name="rT", tag="abt")
            for h in (0, 1):
                sl = slice(h * HW_, (h + 1) * HW_)
                V.tensor_tensor(T16[:, sl], ma_f[:, sl], t_guid[s][:, sl],
                                Alu.mult)
                V.tensor_tensor(T16[:, sl], T16[:, sl], mb_f[:, sl], Alu.add)
                nc.scalar.copy(T_t[:, sl], T16[:, sl])
                V.reciprocal_approx_fast(out=rT[:, sl], in_=T_t[:, sl])
                if s == 0:
                    nc.scalar.copy(poolT[:, sl], rT[:, sl])
            rmul = poolT if s == 0 else rT

            for chn in range(3):
                d_t = dout.tile([128, NW], bf16, name=f"d{chn}", tag="dout")
                yv = y_ext[s, chn].rearrange("(c p) w -> p c w", p=128)
                for h in (0, 1):
                    sl = slice(h * HW_, (h + 1) * HW_)
                    nc.scalar.activation(d_t[:, sl], x16[s][chn][:, sl],
                                         Act.Identity,
                                         bias=chsc[:, 6 + chn:7 + chn],
                                         scale=0.5)
                    V.tensor_tensor(d_t[:, sl], d_t[:, sl], rmul[:, sl],
                                    Alu.mult)
                    V.tensor_scalar(d_t[:, sl], d_t[:, sl],
                                    chsc[:, chn:chn + 1], 0.0,
                                    Alu.add, Alu.add)
                    nc.gpsimd.dma_start(out=yv[:, 2 * h:2 * h + 2, :],
                                        in_=cview(d_t)[:, 2 * h:2 * h + 2, :])

        # ================================================== emission order
        f_load(0)
        f_load(1)
        load_consts()

        # dark1 for sample 0 only; sample 1's pool runs inside s0's secant
        f_chanmin_hpool(0, False)
        t_fwd(0)
        f_guid(0)
        f_hpoolT(0)
        t_back(0)

        f_secant_init(0)
        f_secant_init(1)
        fills = [lambda: f_chanmin_hpool(1, False),
                 lambda: t_fwd(1),
                 lambda: f_guid(1),
                 lambda: f_hpoolT(1),
                 lambda: t_back(1)]
        fills += list(prep_ops(0)) + list(prep_ops(1))
        pi = 0

        def drain_prep(n=1):
            nonlocal pi
            for _ in range(n):
                if pi < len(fills):
                    fills[pi]()
                    pi += 1

        # sample-0 secant units, with sample-1's pool + preps as fill;
        # sample-1 secant lags by 3 units (its pooled dark is ready then)
        q0 = [lambda w=w: f_count0(0, w) for w in (0, 1)]
        q0 += [lambda r=r: f_secant_round(0, r) for r in range(SECANT_ROUNDS)]
        q1 = [lambda w=w: f_count0(1, w) for w in (0, 1)]
        q1 += [lambda r=r: f_secant_round(1, r) for r in range(SECANT_ROUNDS)]
        q0[0]()
        drain_prep(3)
        q0[1]()
        drain_prep(2)
        for i in range(2, len(q0)):
            q1[i - 2]()
            q0[i]()
            drain_prep()
        f_msums(0)
        q1[-2]()
        f_bandprep(0)
        q1[-1]()
        drain_prep()
        f_bandsums(0)
        f_msums(1)
        f_bandprep(1)
        drain_prep()
        f_afold(0)
        f_bandsums(1)
        drain_prep(len(fills))
        f_afold(1)
        f_amath(0)
        f_amath(1)
        f_chsc(0)
        f_chsc(1)

        # dark2 phase with backend_head(0) interleaved after sample 0's pool
        f_chanmin_hpool(0, True)
        t_fwd(0)
        f_chanmin_hpool(1, True)
        f_hpoolT(0)
        t_back(0)
        backend_head(0)
        t_fwd(1)
        f_hpoolT(1)
        t_back(1)
        backend_mid(0)
        backend_head(1)
        backend_tail(0)
        backend_mid(1)
        backend_tail(1)

    nc.compile()
    return nc


def _get_program():
    if "nc" not in _CACHE:
        _CACHE["nc"] = _build()
    return _CACHE["nc"]


def kernel(x: np.ndarray) -> np.ndarray:
    from concourse.bass_utils import run_bass_kernel_spmd
    x = np.ascontiguousarray(np.asarray(x, dtype=np.float32))
    assert x.shape == (16, 3, H, W), x.shape
    nc = _get_program()
    consts = _host_consts()
    in_maps = [{"x": x[2 * i:2 * i + 2], **consts} for i in range(8)]
    res = run_bass_kernel_spmd(nc, in_maps, list(range(8)))
    out = np.concatenate([res.results[i]["y"] for i in range(8)], axis=0)
    return out.astype(np.float32)


# revision 65
# speedup vs baseline: 1.0603x; 1.0115x over previous
"""Dark-Channel-Prior dehazing (DCPGenerator) Trainium2 Bass kernel, v9.

v8 -> v9: the guided filter runs as a fast-guided-filter at 2x subsample
(256x256): all six box filters (I, II, p, Ip, a, b), the cov/var/a/b
math, and the vbox matmuls operate on 1/4 the pixels with radius-20
bands; mean_a/mean_b are bilinearly upsampled (PE matmuls for rows, DVE
for columns) and T = mean_a*I + mean_b is applied at full resolution.
Subsampling of guid / pooled-dark runs on the PE with selection
matrices.  Dark channel, top-k secant, and A estimation stay full-res.
"""
import numpy as np
from contextlib import ExitStack

H = 512
W = 512
NCHUNK = 4
CW = 512
NW = NCHUNK * CW            # 2048
PADW = 526                  # 7 | 512 | 7
WIN_PAD = 7
RADIUS = 40
# sub-grid (fast guided filter, s=2)
HS = 256
WS = 256
NCS = 2
RS = 20
LEAD = 24                   # leading zeros in sub scan layout (>=RS+1)
SEG = 300                   # WS + 44-zero gap (>= 2*RS+1)
SCN_W = LEAD + NCS * SEG    # 624
NWS = NCS * WS              # 512
EPS = 1e-3
OMEGA = 0.95
TOPN = int(0.01 * H * W)    # 2621
T0 = 0.0055
T1 = 0.0085
BAND = 2e-4
SECANT_ROUNDS = 5

_CACHE = {}


def _host_consts():
    # full-res H-direction box weights are no longer needed; sub-grid ones:
    i = np.arange(HS)
    n1s = np.minimum(i + RS, HS - 1) - np.maximum(i - RS, 0) + 1
    inv_ns = (1.0 / n1s).astype(np.float32)
    k = np.arange(128)[:, None]
    p = np.arange(128)[None, :]
    bands = (np.abs(k - p) <= RS).astype(np.float32)
    bus = (k >= p + 128 - RS).astype(np.float32) / 41.0 / 41.0
    bds = (k <= p - (128 - RS)).astype(np.float32) / 41.0 / 41.0
    bm0s = bands * inv_ns[0:128][None, :] / 41.0
    bm1s = bands * inv_ns[128:256][None, :] / 41.0
    fix40 = np.concatenate([41.0 * inv_ns[0:RS], 41.0 * inv_ns[WS - RS:]])
    fixs = np.tile(fix40[None, :], (128, NCS)).copy()        # [128, 80]
    ident = np.eye(128, dtype=np.float32)
    # row-subsample selection: out q <- full partition 2q (two half matrices)
    selA = np.zeros((128, 128), np.float32)
    selB = np.zeros((128, 128), np.float32)
    for q in range(64):
        selA[2 * q, q] = 1.0
    for q in range(64, 128):
        selB[2 * (q - 64), q] = 1.0
    # row-upsample (bilinear, sub sample i at full row 2i)
    U = {}
    for c in range(NCHUNK):
        for q in range(128):
            r = 128 * c + q
            if r % 2 == 0:
                pairs = [(r // 2, 1.0)]
            else:
                i0 = (r - 1) // 2
                i1 = min(i0 + 1, HS - 1)
                pairs = [(i0, 0.5), (i1, 0.5)] if i1 != i0 else [(i0, 1.0)]
            for i_, wgt in pairs:
                sc, pp_ = divmod(i_, 128)
                U.setdefault((c, sc), np.zeros((128, 128), np.float32))[
                    pp_, q] += wgt
    return {"bm0s": bm0s, "bm1s": bm1s, "bus": bus, "bds": bds,
            "fixs": fixs, "ident": ident, "selA": selA, "selB": selB,
            "u00": U[(0, 0)], "u10": U[(1, 0)], "u11": U[(1, 1)],
            "u21": U[(2, 1)], "u31": U[(3, 1)]}


def _build():
    import concourse.bacc as bacc
    import concourse.tile as tile
    import concourse.bass as bass
    from concourse import mybir

    f32 = mybir.dt.float32
    f32r = mybir.dt.float32r
    bf16 = mybir.dt.bfloat16
    Alu = mybir.AluOpType
    Act = mybir.ActivationFunctionType

    nc = bacc.Bacc("TRN2", target_bir_lowering=False, debug=False, num_devices=8)
    V = nc.vector
    G = nc.gpsimd

    x_ext = nc.dram_tensor("x", [2, 3, H, W], f32, kind="ExternalInput").ap()
    c128_names = ("bm0s", "bm1s", "bus", "bds", "ident", "selA", "selB",
                  "u00", "u10", "u11", "u21", "u31")
    c128_exts = {nm: nc.dram_tensor(nm, [128, 128], f32, kind="ExternalInput").ap()
                 for nm in c128_names}
    fixs_ext = nc.dram_tensor("fixs", [128, NCS * 2 * RS], f32,
                              kind="ExternalInput").ap()
    y_ext = nc.dram_tensor("y", [2, 3, H, W], f32, kind="ExternalOutput").ap()

    def cview(t, width=CW):
        return t.rearrange("p (c w) -> p c w", w=width)

    def fbcast(ap_col, n):
        return bass.AP(tensor=ap_col.tensor, offset=ap_col.offset,
                       ap=[list(p) for p in ap_col.ap[:-1]] + [[0, n]])

    def segview(t, off, c0=0, nch=NCS):
        """[128, nch, WS] view into a [128, SCN_W] sub tile."""
        base = t[:]
        return bass.AP(tensor=base.tensor, offset=base.offset + off + c0 * SEG,
                       ap=[list(base.ap[0]), [SEG, nch], [1, WS]])

    def sview(t):
        return segview(t, LEAD)

    with ExitStack() as ctx:
        tc = ctx.enter_context(tile.TileContext(nc))

        cpool = ctx.enter_context(tc.tile_pool(name="cpool", bufs=1))
        srcp = ctx.enter_context(tc.tile_pool(name="srcp", bufs=1))
        scn = ctx.enter_context(tc.tile_pool(name="scn", bufs=1))
        pp = ctx.enter_context(tc.tile_pool(name="pp", bufs=1))
        cump = ctx.enter_context(tc.tile_pool(name="cump", bufs=2))
        boxes = ctx.enter_context(tc.tile_pool(name="boxes", bufs=1))
        rot = ctx.enter_context(tc.tile_pool(name="rot", bufs=2))
        mrot = ctx.enter_context(tc.tile_pool(name="mrot", bufs=4))
        abt = ctx.enter_context(tc.tile_pool(name="abt", bufs=3))
        sab = ctx.enter_context(tc.tile_pool(name="sab", bufs=3))
        dout = ctx.enter_context(tc.tile_pool(name="dout", bufs=2))
        mfull = ctx.enter_context(tc.tile_pool(name="mfull", bufs=2))
        tiny = ctx.enter_context(tc.tile_pool(name="tiny", bufs=1))
        pbig = ctx.enter_context(tc.tile_pool(name="pbig", bufs=1, space="PSUM"))
        pmid = ctx.enter_context(tc.tile_pool(name="pmid", bufs=2, space="PSUM"))
        psml = ctx.enter_context(tc.tile_pool(name="psml", bufs=1, space="PSUM"))

        # ------------------------------------- constants (loaded after x DMAs)
        cbf = {}
        stage = cpool.tile([128, 128], f32, name="s_band")
        for nm in ("bm0s", "bm1s", "bus", "bds"):
            cbf[nm] = cpool.tile([128, 128], f32r, name=f"c_{nm}")
        for nm in ("ident", "selA", "selB", "u00", "u10", "u11", "u21", "u31"):
            cbf[nm] = cpool.tile([128, 128], bf16, name=f"c_{nm}")
        c_fixs = cpool.tile([128, NCS * 2 * RS], f32, name="c_fixs")
        c_ones128 = cpool.tile([128, 1], f32, name="c_ones128")
        c_ones1x = cpool.tile([1, 128], f32, name="c_ones1x")

        def load_consts():
            for nm in ("bm0s", "bm1s", "bus", "bds", "ident", "selA", "selB",
                       "u00", "u10", "u11", "u21", "u31"):
                nc.sync.dma_start(out=stage[:], in_=c128_exts[nm][:])
                nc.scalar.copy(cbf[nm][:], stage[:])
            nc.sync.dma_start(out=c_fixs[:], in_=fixs_ext[:])
            V.memset(c_ones128[:], 1.0)
            V.memset(c_ones1x[:], 1.0)

        # --------------------------------------------------- persistent tiles
        x16 = [[srcp.tile([128, NW], bf16, name=f"x16_{s}_{c}")
                for c in range(3)] for s in range(2)]
        t_guid = [srcp.tile([128, NW], bf16, name=f"guid{s}") for s in range(2)]
        # sub-grid scan-layout sources (f32): I, p, Ip, II, a, b per sample
        t_is = [scn.tile([128, SCN_W], f32, name=f"is{s}") for s in range(2)]
        t_ps = [scn.tile([128, SCN_W], f32, name=f"ps{s}") for s in range(2)]
        t_ip = [scn.tile([128, SCN_W], f32, name=f"ip{s}") for s in range(2)]
        t_ii = [scn.tile([128, SCN_W], f32, name=f"ii{s}") for s in range(2)]
        mxp = pp.tile([128, NCHUNK * PADW], bf16, name="mxp")
        w1 = pp.tile([128, NCHUNK * PADW], bf16, name="w1")
        uhTp = pp.tile([128, NCHUNK * PADW], bf16, name="uhTp")
        poolT = pp.tile([128, NW], bf16, name="poolT")
        uh = [pp.tile([128, NW], bf16, name=f"uh{s}") for s in range(2)]
        mean_Is = [boxes.tile([128, NWS], f32, name=f"meanIs{s}")
                   for s in range(2)]
        rec_s = [boxes.tile([128, NWS], f32, name=f"recs{s}") for s in range(2)]

        junkt = pp.tile([128, NW], bf16, name="junkt")
        junk = junkt[:, 0:NW]
        junk_c = junk.rearrange("p (c w) -> p c w", w=CW)

        # zero the sub scan-layout gaps once
        for t in (t_is[0], t_is[1], t_ps[0], t_ps[1], t_ip[0], t_ip[1],
                  t_ii[0], t_ii[1]):
            V.memset(t[:, 0:LEAD], 0.0)
            for c in range(NCS):
                V.memset(t[:, LEAD + c * SEG + WS: LEAD + (c + 1) * SEG], 0.0)

        # ---------------------------------------------------------- helpers
        def interior(t):
            return cview(t, PADW)[:, :, WIN_PAD:WIN_PAD + CW]

        def memset_pads(t, eng):
            v = cview(t, PADW)
            for c in range(NCHUNK):
                eng.memset(v[:, c, 0:WIN_PAD], 1.0)
                eng.memset(v[:, c, PADW - WIN_PAD:PADW], 1.0)

        def hpool(dst, padded, scratch):
            a = cview(padded, PADW)
            b = cview(scratch, PADW)
            d = cview(dst)
            V.tensor_tensor(b[:, :, 0:525], a[:, :, 0:525], a[:, :, 1:526], Alu.min)
            V.tensor_tensor(a[:, :, 0:523], b[:, :, 0:523], b[:, :, 2:525], Alu.min)
            V.tensor_tensor(b[:, :, 0:519], a[:, :, 0:519], a[:, :, 4:523], Alu.min)
            V.tensor_tensor(d[:, 0:NCHUNK, :], b[:, :, 0:512], b[:, :, 7:519],
                            Alu.min)

        def transpose_blocks(dst_ap, src_flat, on_dve=False):
            sv = cview(src_flat)
            pt = pbig.tile([128, NW], bf16, name="pt", tag="ptp")
            for co in range(NCHUNK):
                for ci in range(NCHUNK):
                    nc.tensor.transpose(
                        pt[:, co * CW + ci * 128: co * CW + (ci + 1) * 128],
                        sv[:, ci, co * 128:(co + 1) * 128], cbf["ident"][:])
            if on_dve:
                V.tensor_copy(dst_ap, cview(pt)[:, :, :])
            else:
                nc.scalar.copy(dst_ap, cview(pt)[:, :, :])

        def t_fwd(s, on_dve=False):
            memset_pads(uhTp, G)
            iv = cview(uhTp, PADW)
            transpose_blocks(iv[:, :, WIN_PAD:WIN_PAD + CW], uh[s], on_dve)

        def t_back(s, on_dve=False):
            transpose_blocks(cview(uh[s])[:, :, :], poolT, on_dve)

        # ------------------------------------------------ sub-grid helpers
        def pe_sub(dst_seg_ap, src_full, scale=1.0, bias=0.0):
            """dst (sub scan-layout data view) <- src_full[::2,::2]*scale+bias."""
            sv = cview(src_full)
            ps = pmid.tile([128, NWS], f32, name="subps", tag="pmid")
            for cs in range(NCS):
                psc = ps[:, cs * WS:(cs + 1) * WS]
                nc.tensor.matmul(psc, cbf["selA"][:],
                                 sv[:, 2 * cs, 0:CW:2], start=True, stop=False)
                nc.tensor.matmul(psc, cbf["selB"][:],
                                 sv[:, 2 * cs + 1, 0:CW:2], start=False,
                                 stop=True)
            if scale == 1.0 and bias == 0.0:
                nc.scalar.copy(dst_seg_ap, cview(ps, WS)[:, :, :])
            else:
                nc.scalar.activation(dst_seg_ap, cview(ps, WS)[:, :, :],
                                     Act.Copy, bias=bias, scale=scale)

        def pe_sub_T(dst_seg_ap, scale, bias):
            """subsample from poolT (transposed pooled image) + re-transpose:
            avoids the full-res back-transpose of the dark2 pool."""
            pv = cview(poolT)
            ps = pmid.tile([128, NWS], f32, name="subTps", tag="pmid")
            for cs in range(NCS):
                psc = ps[:, cs * WS:(cs + 1) * WS]
                nc.tensor.matmul(psc, cbf["selA"][:],
                                 pv[:, 2 * cs, 0:CW:2], start=True, stop=False)
                nc.tensor.matmul(psc, cbf["selB"][:],
                                 pv[:, 2 * cs + 1, 0:CW:2], start=False,
                                 stop=True)
            tT = dout.tile([128, NWS], bf16, name="tT", tag="dout")
            nc.scalar.copy(tT[:], ps[:])
            ps2 = pmid.tile([128, NWS], bf16, name="subT2", tag="pmid")
            tv = cview(tT, WS)
            for ch in range(NCS):
                for cw in range(NCS):
                    nc.tensor.transpose(
                        ps2[:, ch * WS + cw * 128: ch * WS + (cw + 1) * 128],
                        tv[:, cw, ch * 128:(ch + 1) * 128], cbf["ident"][:])
            nc.scalar.activation(dst_seg_ap, cview(ps2, WS)[:, :, :],
                                 Act.Copy, bias=bias, scale=scale)

        def hbox_s(hb_t, src_t):
            cum = cump.tile([128, SCN_W], f32, name="cum", tag="cum")
            for c in range(NCS):
                V.tensor_tensor_scan(cum[:, c * SEG:(c + 1) * SEG],
                                     src_t[:, c * SEG:(c + 1) * SEG],
                                     fbcast(c_ones128[:, 0:1], SEG), 0.0,
                                     Alu.add, Alu.bypass)
            V.tensor_tensor(cview(hb_t, WS)[:, :, :],
                            segview(cum, LEAD + RS),
                            segview(cum, LEAD - RS - 1), Alu.subtract)

        def vbox_s(dst, src):
            sv = cview(src, WS)
            ps = pmid.tile([128, NWS], f32, name="vps", tag="pmid")
            r0 = ps[:, 0:WS]
            r1 = ps[:, WS:NWS]
            nc.tensor.matmul(r0, cbf["bm0s"][:], sv[:, 0, :], start=True,
                             stop=False)
            nc.tensor.matmul(r0, cbf["bds"][:], sv[:, 1, :], start=False,
                             stop=True)
            nc.tensor.matmul(r1, cbf["bm1s"][:], sv[:, 1, :], start=True,
                             stop=False)
            nc.tensor.matmul(r1, cbf["bus"][:], sv[:, 0, :], start=False,
                             stop=True)
            nc.scalar.copy(dst[:], ps[:])
            db = dst[:]
            edges = bass.AP(tensor=db.tensor, offset=db.offset,
                            ap=[list(db.ap[0]), [WS, NCS],
                                [WS - RS, 2], [1, RS]])
            fb = c_fixs[:]
            fv = bass.AP(tensor=fb.tensor, offset=fb.offset,
                         ap=[list(fb.ap[0]), [2 * RS, NCS], [RS, 2], [1, RS]])
            V.tensor_tensor(edges, edges, fv, Alu.mult)

        def upsample(dst_full_bf16, src_sub):
            """bilinear 2x upsample [128, 2x256] f32 -> [128, 4x512] bf16."""
            wide = mrot.tile([128, NCS * CW], bf16, name="wide", tag="wide",
                             bufs=2)
            wv = cview(wide)
            sv = cview(src_sub, WS)
            # W-upsample at sub rows
            wide_e = bass.AP(tensor=wv.tensor, offset=wv.offset,
                             ap=[list(wv.ap[0]), [CW, NCS], [2, WS]])
            V.tensor_copy(wide_e, sv[:, :, :])
            wide_o = bass.AP(tensor=wv.tensor, offset=wv.offset + 1,
                             ap=[list(wv.ap[0]), [CW, NCS], [2, WS - 1]])
            V.tensor_tensor(wide_o, sv[:, :, 0:WS - 1], sv[:, :, 1:WS], Alu.add)
            V.tensor_scalar(wide_o, wide_o, 0.5, 0.0, Alu.mult, Alu.add)
            lastc = bass.AP(tensor=wv.tensor, offset=wv.offset + CW - 1,
                            ap=[list(wv.ap[0]), [CW, NCS], [1, 1]])
            V.tensor_copy(lastc, sv[:, :, WS - 1:WS])
            # H-upsample via PE
            ps = pbig.tile([128, NW], f32, name="ups", tag="ptp")
            for c, mats in enumerate((
                    (("u00", 0),), (("u10", 0), ("u11", 1)),
                    (("u21", 1),), (("u31", 1),))):
                psc = ps[:, c * CW:(c + 1) * CW]
                for i, (nm, sc) in enumerate(mats):
                    nc.tensor.matmul(psc, cbf[nm][:], wv[:, sc, :],
                                     start=(i == 0), stop=(i == len(mats) - 1))
            nc.scalar.copy(dst_full_bf16[:], ps[:])

        # ---------------------------------------------- per-sample frontend
        ST = [dict(), dict()]

        def f_load(s):
            for chn in range(3):
                src = x_ext[s, chn].rearrange("(c p) w -> p c w", p=128)
                nc.gpsimd.dma_start(out=cview(x16[s][chn])[:, :, :], in_=src)

        def f_guid(s):
            gt = t_guid[s]
            tg = dout.tile([128, NW], bf16, name=f"gt{s}", tag="dout")
            tb = dout.tile([128, NW], bf16, name=f"bt{s}", tag="dout")
            nc.scalar.activation(gt[:], x16[s][0][:], Act.Copy,
                                 bias=0.5, scale=0.14945)
            nc.scalar.activation(tg[:], x16[s][1][:], Act.Copy,
                                 bias=0.0, scale=0.2935)
            nc.scalar.activation(tb[:], x16[s][2][:], Act.Copy,
                                 bias=0.0, scale=0.057)
            V.tensor_tensor(gt[:], gt[:], tg[:], Alu.add)
            V.tensor_tensor(gt[:], gt[:], tb[:], Alu.add)

        def f_chanmin_hpool(s, second):
            memset_pads(mxp, G)
            if not second:
                a0, a1, a2 = x16[s]
                V.tensor_tensor(interior(mxp), cview(a0)[:, :, :],
                                cview(a1)[:, :, :], Alu.min)
                V.tensor_tensor(interior(mxp), interior(mxp),
                                cview(a2)[:, :, :], Alu.min)
            else:
                chsc = ST[s]["chsc"]
                ytmp = junk
                nc.scalar.activation(interior(mxp), x16[s][0][:], Act.Identity,
                                     bias=chsc[:, 3:4], scale=chsc[:, 3:4])
                nc.scalar.activation(ytmp, x16[s][1][:], Act.Identity,
                                     bias=chsc[:, 4:5], scale=chsc[:, 4:5])
                V.tensor_tensor(interior(mxp), interior(mxp), junk_c, Alu.min)
                nc.scalar.activation(ytmp, x16[s][2][:], Act.Identity,
                                     bias=chsc[:, 5:6], scale=chsc[:, 5:6])
                V.tensor_tensor(interior(mxp), interior(mxp), junk_c, Alu.min)
            hpool(uh[s], mxp, w1)

        def f_hpoolT(s):
            hpool(poolT, uhTp, w1)

        def dark_phase(second):
            f_chanmin_hpool(0, second)
            t_fwd(0)
            f_chanmin_hpool(1, second)
            if not second:
                f_guid(0)
            f_hpoolT(0)
            t_back(0)
            t_fwd(1)
            if not second:
                f_guid(1)
            f_hpoolT(1)
            t_back(1)

        # ------------------------------------------------------- secant/topk
        def f_secant_init(s):
            st = ST[s]
            st["acc8"] = tiny.tile([128, 8], f32, name=f"acc8{s}", tag=f"acc8{s}")
            V.memset(st["acc8"][:], 0.0)
            st["thr"] = tiny.tile([128, 1], f32, name=f"thr{s}", tag=f"thr{s}")
            st["scal"] = tiny.tile([1, 16], f32, name=f"scal{s}", tag=f"scal{s}")
            V.memset(st["scal"][:], 0.0)
            V.memset(st["scal"][:, 0:1], T0)
            V.memset(st["scal"][:, 2:3], T1)

        def count_into(s, col, sub=False):
            st = ST[s]
            u, acc8, thr = uh[s], st["acc8"], st["thr"]
            uv = cview(u)
            if sub:
                V.tensor_scalar(junk_c[:, 0:2, 0:256],
                                uv[:, 0:NCHUNK:2, 0:CW:2], thr[:], 0.0,
                                Alu.is_gt, Alu.add,
                                accum_out=acc8[:, col:col + 1])
            else:
                V.tensor_scalar(junk, u[:, 0:NW], thr[:], 0.0,
                                Alu.is_gt, Alu.add,
                                accum_out=acc8[:, col:col + 1])
            fps = psml.tile([1, 1], f32, name=f"fold{s}", tag=f"fold{s}")
            nc.tensor.matmul(fps[:], c_ones128[:], acc8[:, col:col + 1],
                             start=True, stop=True)
            return fps

        def bcast_thr(s, src_col):
            st = ST[s]
            bp = psml.tile([128, 1], f32, name=f"thrps{s}", tag=f"fold{s}")
            nc.tensor.matmul(bp[:], c_ones1x[:], src_col, start=True, stop=True)
            nc.scalar.copy(st["thr"][:], bp[:])

        def f_count0(s, which):
            scal = ST[s]["scal"]
            col = 0 if which == 0 else 2
            bcast_thr(s, scal[0:1, col:col + 1])
            f = count_into(s, 0, sub=True)
            nc.scalar.copy(scal[:, col + 1:col + 2], f[:])

        def f_secant_round(s, rnd):
            scal = ST[s]["scal"]
            full = rnd >= SECANT_ROUNDS - 2
            if rnd == SECANT_ROUNDS - 2:
                V.tensor_scalar(scal[:, 1:2], scal[:, 1:2], 4.0, 0.0,
                                Alu.mult, Alu.add)
                V.tensor_scalar(scal[:, 3:4], scal[:, 3:4], 4.0, 0.0,
                                Alu.mult, Alu.add)
            V.tensor_tensor(scal[:, 4:5], scal[:, 3:4], scal[:, 1:2], Alu.subtract)
            V.tensor_scalar(scal[:, 8:9], scal[:, 4:5], -1.0, 0.0, Alu.mult, Alu.add)
            V.tensor_tensor(scal[:, 4:5], scal[:, 4:5], scal[:, 8:9], Alu.max)
            V.tensor_scalar(scal[:, 4:5], scal[:, 4:5], 1.0, 0.0, Alu.max, Alu.add)
            V.tensor_tensor(scal[:, 5:6], scal[:, 2:3], scal[:, 0:1], Alu.subtract)
            V.tensor_scalar(scal[:, 8:9], scal[:, 5:6], -1.0, 0.0, Alu.mult, Alu.add)
            V.tensor_tensor(scal[:, 5:6], scal[:, 5:6], scal[:, 8:9], Alu.max)
            V.reciprocal(scal[:, 8:9], scal[:, 4:5])
            V.tensor_tensor(scal[:, 5:6], scal[:, 5:6], scal[:, 8:9], Alu.mult)
            V.tensor_scalar(scal[:, 6:7], scal[:, 3:4], 1.0,
                            -float(TOPN) if full else -TOPN / 4.0,
                            Alu.mult, Alu.add)
            V.tensor_tensor(scal[:, 6:7], scal[:, 6:7], scal[:, 5:6], Alu.mult)
            V.tensor_copy(scal[:, 0:1], scal[:, 2:3])
            V.tensor_copy(scal[:, 1:2], scal[:, 3:4])
            V.tensor_tensor(scal[:, 2:3], scal[:, 2:3], scal[:, 6:7], Alu.add)
            bcast_thr(s, scal[0:1, 2:3])
            f = count_into(s, 0, sub=not full)
            nc.scalar.copy(scal[:, 3:4], f[:])

        def f_msums(s):
            st = ST[s]
            u, acc8, thr = uh[s], st["acc8"], st["thr"]
            V.tensor_scalar(junk, u[:, 0:NW], thr[:], 0.0,
                            Alu.is_gt, Alu.bypass)
            mbufs = (poolT[:], uhTp[:, 0:NW], mxp[:, 0:NW])
            for chn, xt in enumerate(x16[s]):
                if chn == 0:
                    # one channel per group stays on DVE: shortens the
                    # ACT accumulation chain that gates afold
                    V.scalar_tensor_tensor(mbufs[0], u[:, 0:NW], thr[:],
                                           xt[:], Alu.is_gt, Alu.mult,
                                           accum_out=acc8[:, 1:2])
                    continue
                mb = mbufs[chn % 3]
                V.tensor_tensor(mb, junk, xt[:], Alu.mult)
                nc.scalar.activation(mb, mb, Act.Copy,
                                     accum_out=acc8[:, 1 + chn:2 + chn])

        def f_bandprep(s):
            st = ST[s]
            scal = st["scal"]
            V.tensor_scalar(scal[:, 7:8], scal[:, 2:3], 1.0, -BAND,
                            Alu.mult, Alu.add)
            bcast_thr(s, scal[0:1, 7:8])

        def f_bandsums(s):
            st = ST[s]
            u, acc8, thr = uh[s], st["acc8"], st["thr"]
            V.tensor_scalar(junk, u[:, 0:NW], thr[:], 0.0,
                            Alu.is_gt, Alu.bypass)
            nc.scalar.activation(poolT[:], junk, Act.Copy,
                                 accum_out=acc8[:, 4:5])
            mbufs = (poolT[:], uhTp[:, 0:NW], mxp[:, 0:NW])
            for chn, xt in enumerate(x16[s]):
                if chn == 0:
                    V.scalar_tensor_tensor(mbufs[0], u[:, 0:NW], thr[:],
                                           xt[:], Alu.is_gt, Alu.mult,
                                           accum_out=acc8[:, 5:6])
                    continue
                mb = mbufs[chn % 3]
                V.tensor_tensor(mb, junk, xt[:], Alu.mult)
                nc.scalar.activation(mb, mb, Act.Copy,
                                     accum_out=acc8[:, 5 + chn:6 + chn])

        def f_afold(s):
            st = ST[s]
            tps = psml.tile([1, 8], f32, name=f"totps{s}", tag=f"fold{s}")
            nc.tensor.matmul(tps[:], c_ones128[:], st["acc8"][:],
                             start=True, stop=True)
            tot = tiny.tile([1, 8], f32, name=f"tot{s}", tag=f"tot{s}")
            nc.scalar.copy(tot[:], tps[:])
            st["tot"] = tot

        def f_amath(s):
            st = ST[s]
            tot = st["tot"]
            am = tiny.tile([1, 12], f32, name=f"am{s}", tag=f"am{s}")
            V.tensor_tensor(am[:, 0:3], tot[:, 5:8], tot[:, 1:4], Alu.subtract)
            V.tensor_tensor(am[:, 11:12], tot[:, 4:5], tot[:, 0:1], Alu.subtract)
            V.tensor_scalar(am[:, 11:12], am[:, 11:12], 1.0, 0.0, Alu.max, Alu.add)
            V.reciprocal(am[:, 10:11], am[:, 11:12])
            V.tensor_tensor(am[:, 0:3], am[:, 0:3], fbcast(am[:, 10:11], 3), Alu.mult)
            V.tensor_scalar(am[:, 9:10], tot[:, 0:1], -1.0, float(TOPN),
                            Alu.mult, Alu.add)
            V.tensor_tensor(am[:, 0:3], am[:, 0:3], fbcast(am[:, 9:10], 3), Alu.mult)
            V.tensor_tensor(am[:, 0:3], am[:, 0:3], tot[:, 1:4], Alu.add)
            V.tensor_scalar(am[:, 0:3], am[:, 0:3], 1.0 / TOPN, 0.0, Alu.mult, Alu.add)
            V.tensor_scalar(am[:, 3:6], am[:, 0:3], 1.0, 1.0, Alu.mult, Alu.add)
            V.reciprocal(am[:, 3:6], am[:, 3:6])
            V.tensor_scalar(am[:, 0:3], am[:, 0:3], 0.5, 0.5, Alu.mult, Alu.add)
            V.tensor_scalar(am[:, 6:9], am[:, 0:3], -1.0, 0.5, Alu.mult, Alu.add)
            st["am"] = am

        def f_chsc(s):
            st = ST[s]
            st["chsc"] = tiny.tile([128, 9], f32, name=f"chsc{s}",
                                   tag=f"chsc{s}")
            bp = psml.tile([128, 9], f32, name=f"chps{s}", tag=f"fold{s}")
            nc.tensor.matmul(bp[:], c_ones1x[:], st["am"][0:1, 0:9],
                             start=True, stop=True)
            nc.scalar.copy(st["chsc"][:], bp[:])

        # ------------------------------------------- guidance-only box prep
        def prep_ops(s):
            yield lambda: pe_sub(sview(t_is[s]), t_guid[s])
            yield lambda: nc.scalar.activation(sview(t_ii[s]), sview(t_is[s]),
                                               Act.Square)
            hbI = [None]
            hbII = [None]
            mII = [None]

            def scanI():
                hbI[0] = rot.tile([128, NWS], f32r, name="hbI", tag="hbx")
                hbox_s(hbI[0], t_is[s])
            yield scanI
            yield lambda: vbox_s(mean_Is[s], hbI[0])

            def scanII():
                hbII[0] = rot.tile([128, NWS], f32r, name="hbII", tag="hbx")
                hbox_s(hbII[0], t_ii[s])
            yield scanII

            def vboxII():
                mII[0] = mrot.tile([128, NWS], f32, name="mII", tag="mpx")
                vbox_s(mII[0], hbII[0])
            yield vboxII

            def varrec():
                sq = sab.tile([128, NWS], f32, name="sq", tag="sab")
                nc.scalar.activation(sq[:], mean_Is[s][:], Act.Square)
                V.scalar_tensor_tensor(sq[:], mII[0][:], EPS, sq[:],
                                       Alu.add, Alu.subtract)
                V.reciprocal_approx_fast(out=rec_s[s][:], in_=sq[:])
            yield varrec

        # ---------------------------------------------------------- backend
        BK = [dict(), dict()]

        def backend_head(s):
            pe_sub(sview(t_ps[s]), uh[s], scale=-OMEGA, bias=1.0)
            V.tensor_tensor(sview(t_ip[s]), sview(t_is[s]), sview(t_ps[s]),
                            Alu.mult)
            hb_p = rot.tile([128, NWS], f32r, name="hb_p", tag="hbx")
            hbox_s(hb_p, t_ps[s])
            mean_p = mrot.tile([128, NWS], f32, name="mean_p", tag="mpx")
            vbox_s(mean_p, hb_p)
            hb_ip = rot.tile([128, NWS], f32r, name="hb_ip", tag="hbx")
            hbox_s(hb_ip, t_ip[s])
            mean_Ip = mrot.tile([128, NWS], f32, name="mean_Ip", tag="mpx")
            vbox_s(mean_Ip, hb_ip)
            BK[s]["mp"], BK[s]["mip"] = mean_p, mean_Ip

        def backend_mid(s):
            mean_p, mean_Ip = BK[s]["mp"], BK[s]["mip"]
            tmp = sab.tile([128, NWS], f32, name="tmp", tag="sab")
            V.tensor_tensor(tmp[:], mean_Is[s][:], mean_p[:], Alu.mult)
            cov = sab.tile([128, NWS], f32, name="cov", tag="sab")
            V.tensor_tensor(cov[:], mean_Ip[:], tmp[:], Alu.subtract)
            a_v = sview(t_ip[s])          # overwrite Ip (dead) with a
            V.tensor_tensor(a_v, cview(cov, WS)[:, :, :],
                            cview(rec_s[s], WS)[:, :, :], Alu.mult)
            t2 = sab.tile([128, NWS], f32, name="t2", tag="sab")
            V.tensor_tensor(cview(t2, WS)[:, :, :], a_v,
                            cview(mean_Is[s], WS)[:, :, :], Alu.mult)
            b_v = sview(t_ps[s])          # overwrite p (dead) with b
            V.tensor_tensor(b_v, cview(mean_p, WS)[:, :, :],
                            cview(t2, WS)[:, :, :], Alu.subtract)

            hba = rot.tile([128, NWS], f32r, name="hba", tag="hbx")
            hbox_s(hba, t_ip[s])
            mean_a = mrot.tile([128, NWS], f32, name="mean_a", tag="mpx")
            vbox_s(mean_a, hba)
            hbb = rot.tile([128, NWS], f32r, name="hbb", tag="hbx")
            hbox_s(hbb, t_ps[s])
            mean_b = mrot.tile([128, NWS], f32, name="mean_b", tag="mpx")
            vbox_s(mean_b, hbb)
            ma_f = mfull.tile([128, NW], bf16, name="ma_f", tag="mf")
            upsample(ma_f, mean_a)
            mb_f = mfull.tile([128, NW], bf16, name="mb_f", tag="mf")
            upsample(mb_f, mean_b)
            BK[s]["ma"], BK[s]["mb"] = ma_f, mb_f

        def backend_tail(s):
            chsc = ST[s]["chsc"]
            ma_f, mb_f = BK[s]["ma"], BK[s]["mb"]
            HW_ = NW // 2
            T16 = dout.tile([128, NW], bf16, name="T16", tag="dout")
            T_t = abt.tile([128, NW], f32, name="T_t", tag="abt")
            rT = abt.tile([128, NW], f32, name="rT", tag="abt")
            for h in (0, 1):
                sl = slice(h * HW_, (h + 1) * HW_)
                V.tensor_tensor(T16[:, sl], ma_f[:, sl], t_guid[s][:, sl],
                                Alu.mult)
                V.tensor_tensor(T16[:, sl], T16[:, sl], mb_f[:, sl], Alu.add)
                nc.scalar.copy(T_t[:, sl], T16[:, sl])
                V.reciprocal_approx_fast(out=rT[:, sl], in_=T_t[:, sl])
                if s == 0:
                    nc.scalar.copy(poolT[:, sl], rT[:, sl])
            rmul = poolT if s == 0 else rT

            for chn in range(3):
                d_t = dout.tile([128, NW], bf16, name=f"d{chn}", tag="dout")
                yv = y_ext[s, chn].rearrange("(c p) w -> p c w", p=128)
                for h in (0, 1):
                    sl = slice(h * HW_, (h + 1) * HW_)
                    nc.scalar.activation(d_t[:, sl], x16[s][chn][:, sl],
                                         Act.Identity,
                                         bias=chsc[:, 6 + chn:7 + chn],
                                         scale=0.5)
                    V.tensor_tensor(d_t[:, sl], d_t[:, sl], rmul[:, sl],
                                    Alu.mult)
                    V.tensor_scalar(d_t[:, sl], d_t[:, sl],
                                    chsc[:, chn:chn + 1], 0.0,
                                    Alu.add, Alu.add)
                    nc.gpsimd.dma_start(out=yv[:, 2 * h:2 * h + 2, :],
                                        in_=cview(d_t)[:, 2 * h:2 * h + 2, :])

        # ================================================== emission order
        f_load(0)
        f_load(1)
        load_consts()

        # dark1 for sample 0 only; sample 1's pool runs inside s0's secant
        f_chanmin_hpool(0, False)
        t_fwd(0)
        f_guid(0)
        f_hpoolT(0)
        t_back(0)

        f_secant_init(0)
        f_secant_init(1)
        fills = [lambda: f_chanmin_hpool(1, False),
                 lambda: t_fwd(1),
                 lambda: f_guid(1),
                 lambda: f_hpoolT(1),
                 lambda: t_back(1)]
        fills += list(prep_ops(0)) + list(prep_ops(1))
        pi = 0

        def drain_prep(n=1):
            nonlocal pi
            for _ in range(n):
                if pi < len(fills):
                    fills[pi]()
                    pi += 1

        # sample-0 secant units, with sample-1's pool + preps as fill;
        # sample-1 secant lags by 3 units (its pooled dark is ready then)
        q0 = [lambda w=w: f_count0(0, w) for w in (0, 1)]
        q0 += [lambda r=r: f_secant_round(0, r) for r in range(SECANT_ROUNDS)]
        q1 = [lambda w=w: f_count0(1, w) for w in (0, 1)]
        q1 += [lambda r=r: f_secant_round(1, r) for r in range(SECANT_ROUNDS)]
        q0[0]()
        drain_prep(3)
        q0[1]()
        drain_prep(2)
        for i in range(2, len(q0)):
            q1[i - 2]()
            q0[i]()
            drain_prep()
        f_msums(0)
        q1[-2]()
        f_bandprep(0)
        q1[-1]()
        drain_prep()
        f_bandsums(0)
        f_msums(1)
        f_bandprep(1)
        drain_prep()
        f_afold(0)
        f_bandsums(1)
        drain_prep(len(fills))
        f_afold(1)
        f_amath(0)
        f_amath(1)
        f_chsc(0)
        f_chsc(1)

        # dark2 phase with backend_head(0) interleaved after sample 0's pool
        f_chanmin_hpool(0, True)
        t_fwd(0)
        f_chanmin_hpool(1, True)
        f_hpoolT(0)
        t_back(0)
        backend_head(0)
        t_fwd(1)
        f_hpoolT(1)
        t_back(1)
        backend_mid(0)
        backend_head(1)
        backend_tail(0)
        backend_mid(1)
        backend_tail(1)

    nc.compile()
    return nc


def _get_program():
    if "nc" not in _CACHE:
        _CACHE["nc"] = _build()
    return _CACHE["nc"]


def kernel(x: np.ndarray) -> np.ndarray:
    from concourse.bass_utils import run_bass_kernel_spmd
    x = np.ascontiguousarray(np.asarray(x, dtype=np.float32))
    assert x.shape == (16, 3, H, W), x.shape
    nc = _get_program()
    consts = _host_consts()
    in_maps = [{"x": x[2 * i:2 * i + 2], **consts} for i in range(8)]
    res = run_bass_kernel_spmd(nc, in_maps, list(range(8)))
    out = np.concatenate([res.results[i]["y"] for i in range(8)], axis=0)
    return out.astype(np.float32)
